# revision 3
# baseline (speedup 1.0000x reference)
"""Multi-head attention (L=2048, EMB=1024, H=16, D=64) on 8 TRN2 NeuronCores.

Tensor-parallel over heads: core i owns heads {2i, 2i+1} (a 128-row block of
Wq/Wk/Wv and a 128-column block of Wo). Each core computes its two heads'
attention plus its partial output projection; the host sums the 8 partials.

Device-side layout is fully transposed (scores^T = [m, l]) so no on-device
transposes of the score matrix are needed:
  QT[d, l] = (Wq_shard @ q^T)        lhsT = (Wq_shard/8)^T, rhs = q^T
  KT[d, l] = (Wk_shard @ k^T)
  VT[d, m] = (Wv_shard @ v^T)        then XBAR DMA-transpose -> vaug[m, d]
  sT[m, l] = KT_h^T @ QT_h           (per head, contraction d=64)
  pT       = exp(sT) * keepT         (no max-subtraction: |s| <~ 9)
  attnT/Z  = [V_h | 1]^T @ pT        (ones column gives softmax denominator)
  outT     = Wo_shard^T-block @ (attnT / Z)   partial, summed on host

Key optimizations over the straightforward version (from NTFF traces):
- mask ships as fp8 (0.0/1.0) in a chunk-contiguous DRAM layout and is
  cast to bf16 in-flight by the gpsimd SWDGE ring: halves mask HBM reads
  and makes every DMA packet dense.
- output partials ship as f16 (halves output traffic; ~0.05% noise).
- V projection computes VT with 32 big matmuls + 16 XBAR DMA transposes
  instead of 128 tiny matmuls (PE time for V drops ~4x).
- input DMAs are issued in exact consumption order across the three
  rings (scalar/sync HWDGE + gpsimd SWDGE) so the first score matmul
  isn't stuck behind bulk traffic.
- per-(lt,h) softmax-denominator chains (DRAM bounce to spread the
  reciprocal over 8 partitions) and the output projection are deferred
  and drip-fed into the following block's quad stream; the final
  block's epilogue is flushed ASAP to keep the tail short.
- PSUM: 6 banks double-buffered scores, 1 bank attention accumulator,
  1 bank shared by projections + output projection.
"""

import sys

for _p in ("/opt/trn_rl_repo",):
    if _p not in sys.path:
        sys.path.insert(0, _p)

from contextlib import ExitStack

import ml_dtypes
import numpy as np

import concourse.bass as bass
import concourse.tile as tile
from concourse import bacc, mybir
from concourse._compat import with_exitstack
from concourse.bass_utils import run_bass_kernel_spmd

BF16 = mybir.dt.bfloat16
FP8 = mybir.dt.float8e4
F16 = mybir.dt.float16
F32 = mybir.dt.float32
NPBF16 = ml_dtypes.bfloat16
NPFP8 = ml_dtypes.float8_e4m3
NPF16 = np.float16

L = 2048
EMB = 1024
NHEAD = 16
HEAD_DIM = 64
NCORES = 8
HPC = NHEAD // NCORES  # heads per core = 2
ROWS = HPC * HEAD_DIM  # weight rows per core = 128
SCALE = HEAD_DIM ** -0.5

LT = 512               # l-tile (matmul free dim / PSUM bank)
NLT = L // LT          # 4
MT = 128               # m-tile (key-block on partitions)
NMT = L // MT          # 16
ET = 128               # contraction tile over EMB
NET = EMB // ET        # 8
JT = 128               # output-row tile
NJT = EMB // JT        # 8
NBLK = NLT * HPC       # 8 (lt, h) blocks

QUADS = (3, 3, 3, 3, 2, 2)   # m-tiles per exp/mask-mult instruction
QB = 3                        # psc tile m-capacity (PSUM banks per slot)
VROW = 66                     # 64 V cols + ones + pad


@with_exitstack
def _mha_kernel(ctx, tc, outT, qT, kT, vT, wqT, wkT, wvT, woT, maskP):
    nc = tc.nc

    const = ctx.enter_context(tc.tile_pool(name="const", bufs=1))
    ppool = ctx.enter_context(tc.tile_pool(name="ptiles", bufs=4))
    maskp = ctx.enter_context(tc.tile_pool(name="maskp", bufs=3))
    stage = ctx.enter_context(tc.tile_pool(name="stage", bufs=4))
    zpool = ctx.enter_context(tc.tile_pool(name="zpool", bufs=2))
    trp = ctx.enter_context(tc.tile_pool(name="trp", bufs=2))
    psc = ctx.enter_context(tc.tile_pool(name="psc", bufs=2, space="PSUM"))
    psa = ctx.enter_context(tc.tile_pool(name="psa", bufs=1, space="PSUM"))
    pso = ctx.enter_context(tc.tile_pool(name="pso", bufs=1, space="PSUM"))

    # ---- resident input tiles ----
    qTs = const.tile([128, NET, L], BF16, tag="qTs")
    kTs = const.tile([128, NET, L], BF16, tag="kTs")
    vTs = const.tile([128, NET, L], BF16, tag="vTs")
    wqs = const.tile([128, NET, ROWS], BF16, tag="wqs")
    wks = const.tile([128, NET, ROWS], BF16, tag="wks")
    wvs = const.tile([128, NET, ROWS], BF16, tag="wvs")
    wos = const.tile([128, EMB], BF16, tag="wos")  # [hd, j]
    q3 = qT.rearrange("(o p) l -> p o l", p=128)
    k3 = kT.rearrange("(o p) l -> p o l", p=128)
    v3 = vT.rearrange("(o p) l -> p o l", p=128)

    def chunk(eng, dst, src3, lc):
        eng.dma_start(dst[:, :, bass.ts(lc, LT)], src3[:, :, bass.ts(lc, LT)])

    # ---- input DMA issue, consumption order ----
    # scalar HWDGE: Q-projection criticals, then v chunks (VT), then wos
    nc.scalar.dma_start(wqs[:], wqT[:])
    chunk(nc.scalar, qTs, q3, 0)
    nc.scalar.dma_start(wvs[:], wvT[:])
    for lc in range(NLT):
        chunk(nc.scalar, vTs, v3, lc)
    nc.scalar.dma_start(wos[:], woT[:])
    # sync HWDGE: K-projection criticals, then k chunks, then q tail
    nc.sync.dma_start(wks[:], wkT[:])
    for lc in range(NLT):
        chunk(nc.sync, kTs, k3, lc)
    for lc in range(1, NLT):
        chunk(nc.sync, qTs, q3, lc)

    # gpsimd SWDGE: mask chunks (fp8 DRAM -> bf16 SBUF casting DMAs)
    mask_tiles = {}

    def mask_fetch(b, splits):
        lt, h = divmod(b, HPC)
        mc = maskp.tile([128, NMT, LT], BF16, tag="maskc", name=f"maskc_{lt}_{h}")
        mask_tiles[b] = mc
        a = 0
        for n in splits:
            nc.gpsimd.dma_start(
                mc[:, a : a + n, :], maskP[lt, h, :, a : a + n, :]
            )
            a += n

    mask_fetch(0, QUADS)          # per-quad gating for the first block
    mask_fetch(1, (8, 8))

    # ---- working tiles ----
    QTb = const.tile([128, L], BF16, tag="QTb")
    KTb = const.tile([128, L], BF16, tag="KTb")
    VTb = const.tile([128, L], BF16, tag="VTb")
    vaug = const.tile([128, HPC, NMT, VROW], BF16, tag="vaug")
    nc.vector.memset(vaug[:, :, :, HEAD_DIM : HEAD_DIM + 1], 1.0)
    nc.vector.memset(vaug[:, :, :, HEAD_DIM + 1 : VROW], 0.0)
    attnTb = const.tile([128, L], BF16, tag="attnTb")

    def qk_proj(dst, w, x, lc):
        ps = pso.tile([128, LT], F32, tag="pso", name="ps_proj")
        for et in range(NET):
            nc.tensor.matmul(
                ps[:],
                lhsT=w[:, et, :],
                rhs=x[:, et, bass.ts(lc, LT)],
                start=(et == 0),
                stop=(et == NET - 1),
            )
        nc.vector.tensor_copy(out=dst[:, bass.ts(lc, LT)], in_=ps[:])

    def vt_proj(lc):
        qk_proj(VTb, wvs, vTs, lc)
        for mi in range(LT // MT):
            mt = lc * (LT // MT) + mi
            tr = trp.tile([128, MT], BF16, tag="tr", name=f"tr_{mt}")
            nc.scalar.dma_start(
                tr[:], VTb[:, bass.ts(mt, MT)], transpose=True
            )
            for h in range(HPC):
                nc.gpsimd.tensor_copy(
                    out=vaug[:, h, mt, 0:HEAD_DIM],
                    in_=tr[:, bass.ts(h, HEAD_DIM)],
                )

    # ---- deferred epilogue pieces, drip-fed into later quads ----
    zdram = nc.dram_tensor("zdram", [NLT, HPC, LT], F32).ap()
    zidram = nc.dram_tensor("zidram", [NLT, HPC, LT], BF16).ap()
    state = {}
    pending = []  # [ready_quad, fn]
    quad_no = [0]

    def pop_pending(budget=3, flush=False):
        while pending and budget > 0:
            if not flush and pending[0][0] > quad_no[0]:
                break
            pending.pop(0)[1]()
            budget -= 1

    def piece_zstore(lt, h):
        def go():
            nc.sync.dma_start(
                zdram[lt, h][None, :], state[lt, h, "zseg"][:]
            )
        return go

    def piece_zload(lt, h):
        def go():
            zsp = zpool.tile([8, LT // 8], F32, tag="zsp", name=f"zsp_{lt}_{h}")
            nc.sync.dma_start(zsp[:], zdram[lt, h].rearrange("(o p) -> o p", o=8))
            state[lt, h, "zsp"] = zsp
        return go

    def piece_recip(lt, h):
        def go():
            zsp = state[lt, h, "zsp"]
            nc.vector.reciprocal(zsp[:], zsp[:])
            zspb = zpool.tile([8, LT // 8], BF16, tag="zspb", name=f"zspb_{lt}_{h}")
            nc.vector.tensor_copy(out=zspb[:], in_=zsp[:])
            nc.sync.dma_start(
                zidram[lt, h].rearrange("(o p) -> o p", o=8), zspb[:]
            )
        return go

    def piece_zbcast(lt, h):
        def go():
            # full-height tile so the norm's operands share a base partition
            zinvb = zpool.tile(
                [128, LT], BF16, tag="zinvb", name=f"zinvb_{lt}_{h}"
            )
            nc.sync.dma_start(
                zinvb[bass.ts(h, HEAD_DIM), :],
                zidram[lt, h][None, :].to_broadcast((HEAD_DIM, LT)),
            )
            state[lt, h, "zinvb"] = zinvb
        return go

    def piece_norm(lt, h):
        def go():
            ls = bass.ts(lt, LT)
            hd = bass.ts(h, HEAD_DIM)
            nc.vector.tensor_mul(
                out=attnTb[hd, ls],
                in0=attnTb[hd, ls],
                in1=state[lt, h, "zinvb"][hd, :],
            )
        return go

    def piece_outproj(lt, jt):
        def go():
            ls = bass.ts(lt, LT)
            ps = pso.tile([128, LT], F32, tag="pso", name="ps_out")
            nc.tensor.matmul(
                ps[:],
                lhsT=wos[:, bass.ts(jt, JT)],
                rhs=attnTb[:, ls],
                start=True,
                stop=True,
            )
            st = stage.tile([128, LT], F16, tag="st", name="st")
            nc.vector.tensor_copy(out=st[:], in_=ps[:])
            nc.gpsimd.dma_start(outT[bass.ts(jt, JT), ls], st[:])
        return go

    qk_proj(QTb, wqs, qTs, 0)

    # ---- attention blocks ----
    for b in range(NBLK):
        lt, h = divmod(b, HPC)
        ls = bass.ts(lt, LT)
        hd = bass.ts(h, HEAD_DIM)
        if b + 2 < NBLK:
            mask_fetch(b + 2, (NMT,))
        maskc = mask_tiles[b]
        pa = psa.tile([128, LT], F32, tag="psa", name=f"psa_{lt}_{h}")
        mt0 = 0
        prev_attn = None
        chunks_done = [0] if b == 0 else [NLT]
        for qi, qn in enumerate(QUADS):
            if b == 0:
                # interleave K/V projection chunks in consumption order
                need = min(NLT, (mt0 + qn + 3) // (LT // MT))
                while chunks_done[0] < need:
                    c = chunks_done[0]
                    qk_proj(KTb, wks, kTs, c)
                    vt_proj(c)
                    chunks_done[0] += 1
            if b == 1 and 1 <= qi <= 3:
                qk_proj(QTb, wqs, qTs, qi)  # PE filler + needed later
            pop_pending()
            ss = psc.tile([128, QB, LT], F32, tag="psc", name="ss")
            for i in range(qn):
                nc.tensor.matmul(
                    ss[:, i, :],
                    lhsT=KTb[hd, bass.ts(mt0 + i, MT)],
                    rhs=QTb[hd, ls],
                    start=True,
                    stop=True,
                )
            # one-quad software pipeline on PE: the previous quad's attn
            # matmuls are emitted AFTER this quad's scores, so the in-order
            # PE queue never blocks scores behind exp->mask-mult
            if prev_attn is not None:
                prev_attn()
            pT = ppool.tile([128, QB, LT], BF16, tag="pT", name="pT")
            nc.scalar.activation(
                pT[:, :qn, :], ss[:, :qn, :], mybir.ActivationFunctionType.Exp
            )
            nc.vector.tensor_mul(
                out=pT[:, :qn, :],
                in0=pT[:, :qn, :],
                in1=maskc[:, mt0 : mt0 + qn, :],
            )

            def make_attn(mt0=mt0, qn=qn, pT=pT, pa=pa, h=h):
                def go():
                    for i in range(qn):
                        mt = mt0 + i
                        nc.tensor.matmul(
                            pa[:VROW, :],
                            lhsT=vaug[:, h, mt, :],
                            rhs=pT[:, i, :],
                            start=(mt == 0),
                            stop=(mt == NMT - 1),
                        )
                return go

            prev_attn = make_attn()
            mt0 += qn
            quad_no[0] += 1
        prev_attn()
        nc.vector.tensor_copy(out=attnTb[hd, ls], in_=pa[0:HEAD_DIM, :])
        zseg = zpool.tile([1, LT], F32, tag="zseg", name=f"zseg_{lt}_{h}")
        nc.vector.tensor_copy(
            out=zseg[:], in_=pa[HEAD_DIM : HEAD_DIM + 1, :]
        )
        state[lt, h, "zseg"] = zseg
        q0 = quad_no[0]
        pending.append([q0 + 0, piece_zstore(lt, h)])
        pending.append([q0 + 1, piece_zload(lt, h)])
        pending.append([q0 + 2, piece_recip(lt, h)])
        pending.append([q0 + 3, piece_zbcast(lt, h)])
        pending.append([q0 + 4, piece_norm(lt, h)])
        if h == 1:
            for jt in range(NJT):
                pending.append([q0 + 4 + (jt + 1) // 2, piece_outproj(lt, jt)])

    pop_pending(budget=len(pending), flush=True)


_CACHE = {}


def _build():
    if "nc" in _CACHE:
        return _CACHE["nc"]
    nc = bacc.Bacc("TRN2", target_bir_lowering=False, debug=False,
                   num_devices=NCORES)
    qT = nc.dram_tensor("qT", [EMB, L], BF16, kind="ExternalInput").ap()
    kT = nc.dram_tensor("kT", [EMB, L], BF16, kind="ExternalInput").ap()
    vT = nc.dram_tensor("vT", [EMB, L], BF16, kind="ExternalInput").ap()
    wqT = nc.dram_tensor("wqT", [128, NET, ROWS], BF16, kind="ExternalInput").ap()
    wkT = nc.dram_tensor("wkT", [128, NET, ROWS], BF16, kind="ExternalInput").ap()
    wvT = nc.dram_tensor("wvT", [128, NET, ROWS], BF16, kind="ExternalInput").ap()
    woT = nc.dram_tensor("woT", [ROWS, EMB], BF16, kind="ExternalInput").ap()
    maskP = nc.dram_tensor(
        "maskP", [NLT, HPC, 128, NMT, LT], FP8, kind="ExternalInput"
    ).ap()
    outT = nc.dram_tensor("outT", [EMB, L], F16, kind="ExternalOutput").ap()

    with tile.TileContext(nc) as tc:
        _mha_kernel(tc, outT, qT, kT, vT, wqT, wkT, wvT, woT, maskP)
    nc.compile()
    _CACHE["nc"] = nc
    return nc


def _pack_w(w):
    # [ROWS, EMB] -> w.T [EMB, ROWS] -> [128, NET, ROWS] with e = o*128+p
    return np.ascontiguousarray(
        w.T.reshape(NET, 128, ROWS).transpose(1, 0, 2)
    ).astype(NPBF16)


def _pack_mask(keep):
    # keep [HPC, l(query), m(key)] -> keepT [HPC, m, l]
    # -> [NLT, HPC, 128(p), NMT(mo), LT] chunk-contiguous, m = mo*128+p
    keepT = keep.swapaxes(1, 2)
    m5 = keepT.reshape(HPC, NMT, 128, NLT, LT).transpose(3, 0, 2, 1, 4)
    return np.ascontiguousarray(m5).astype(NPFP8)


def _prep_in_maps(q, k, v, mask, Wq, Wk, Wv, Wo):
    qT = np.ascontiguousarray(q.T).astype(NPBF16)
    kT = np.ascontiguousarray(k.T).astype(NPBF16)
    vT = np.ascontiguousarray(v.T).astype(NPBF16)
    in_maps = []
    for c in range(NCORES):
        rows = slice(c * ROWS, (c + 1) * ROWS)
        in_maps.append({
            "qT": qT,
            "kT": kT,
            "vT": vT,
            "wqT": _pack_w(Wq[rows] * SCALE),
            "wkT": _pack_w(Wk[rows]),
            "wvT": _pack_w(Wv[rows]),
            "woT": np.ascontiguousarray(Wo[:, rows].T).astype(NPBF16),
            "maskP": _pack_mask(~mask[c * HPC : (c + 1) * HPC]),
        })
    return in_maps


def run(q, k, v, mask, Wq, Wk, Wv, Wo, **spmd_kwargs):
    nc = _build()
    in_maps = _prep_in_maps(q, k, v, mask, Wq, Wk, Wv, Wo)
    res = run_bass_kernel_spmd(nc, in_maps, list(range(NCORES)), **spmd_kwargs)
    outT = np.zeros((EMB, L), np.float64)
    for r in res.results:
        outT += r["outT"].astype(np.float64)
    out = np.ascontiguousarray(outT.T).astype(np.float32)
    return out, res


def kernel(q, k, v, mask, Wq, Wk, Wv, Wo):
    q, k, v = (np.asarray(x, np.float32) for x in (q, k, v))
    Wq, Wk, Wv, Wo = (np.asarray(x, np.float32) for x in (Wq, Wk, Wv, Wo))
    mask = np.asarray(mask, bool)
    out, _ = run(q, k, v, mask, Wq, Wk, Wv, Wo)
    return out


# revision 9
# speedup vs baseline: 1.0449x; 1.0449x over previous
"""Multi-head attention (L=2048, EMB=1024, H=16, D=64) on 8 TRN2 NeuronCores.

Tensor-parallel over heads: core i owns heads {2i, 2i+1} (a 128-row block of
Wq/Wk/Wv and a 128-column block of Wo). Each core computes its two heads'
attention plus its partial output projection; the host sums the 8 partials.

Device-side layout is fully transposed (scores^T = [m, l]) so no on-device
transposes of the score matrix are needed:
  QT[d, l] = (Wq_shard @ q^T)        lhsT = (Wq_shard/8)^T, rhs = q^T
  KT[d, l] = (Wk_shard @ k^T)
  VT[d, m] = (Wv_shard @ v^T)        then XBAR DMA-transpose -> vaug[m, d]
  sT[m, l] = KT_h^T @ QT_h           (per head, contraction d=64)
  pT       = exp(sT) * keepT         (no max-subtraction: |s| <~ 9)
  attnT/Z  = [V_h | 1]^T @ pT        (ones column gives softmax denominator)
  outT     = Wo_shard^T-block @ (attnT / Z)   partial, summed on host

Key optimizations over the straightforward version (from NTFF traces):
- mask ships as fp8 (0.0/1.0) in a chunk-contiguous DRAM layout and is
  cast to bf16 in-flight by the gpsimd SWDGE ring: halves mask HBM reads
  and makes every DMA packet dense.
- output partials ship as f16 (halves output traffic; ~0.05% noise).
- V projection computes VT with 32 big matmuls + 16 XBAR DMA transposes
  instead of 128 tiny matmuls (PE time for V drops ~4x).
- input DMAs are issued in exact consumption order across the three
  rings (scalar/sync HWDGE + gpsimd SWDGE) so the first score matmul
  isn't stuck behind bulk traffic.
- per-(lt,h) softmax-denominator chains (DRAM bounce to spread the
  reciprocal over 8 partitions) and the output projection are deferred
  and drip-fed into the following block's quad stream; the final
  block's epilogue is flushed ASAP to keep the tail short.
- PSUM: 6 banks double-buffered scores, 1 bank attention accumulator,
  1 bank shared by projections + output projection.
"""

import sys

for _p in ("/opt/trn_rl_repo",):
    if _p not in sys.path:
        sys.path.insert(0, _p)

from contextlib import ExitStack

import ml_dtypes
import numpy as np

import concourse.bass as bass
import concourse.tile as tile
from concourse import bacc, mybir
from concourse._compat import with_exitstack
from concourse.bass_utils import run_bass_kernel_spmd

BF16 = mybir.dt.bfloat16
FP8 = mybir.dt.float8e4
F16 = mybir.dt.float16
F32 = mybir.dt.float32
NPBF16 = ml_dtypes.bfloat16
NPFP8 = ml_dtypes.float8_e4m3
NPF16 = np.float16

L = 2048
EMB = 1024
NHEAD = 16
HEAD_DIM = 64
NCORES = 8
HPC = NHEAD // NCORES  # heads per core = 2
ROWS = HPC * HEAD_DIM  # weight rows per core = 128
SCALE = HEAD_DIM ** -0.5

LT = 512               # l-tile (matmul free dim / PSUM bank)
NLT = L // LT          # 4
MT = 128               # m-tile (key-block on partitions)
NMT = L // MT          # 16
ET = 128               # contraction tile over EMB
NET = EMB // ET        # 8
JT = 128               # output-row tile
NJT = EMB // JT        # 8
NBLK = NLT * HPC       # 8 (lt, h) blocks

QUADS = (3, 3, 3, 3, 2, 2)   # m-tiles per exp/mask-mult instruction
QB = 3                        # psc tile m-capacity (PSUM banks per slot)
VROW = 66                     # 64 V cols + ones + pad


@with_exitstack
def _mha_kernel(ctx, tc, outT, qT, kT, vT, wqT, wkT, wvT, woT, maskP, identT):
    nc = tc.nc

    const = ctx.enter_context(tc.tile_pool(name="const", bufs=1))
    ppool = ctx.enter_context(tc.tile_pool(name="ptiles", bufs=4))
    maskp = ctx.enter_context(tc.tile_pool(name="maskp", bufs=3))
    stage = ctx.enter_context(tc.tile_pool(name="stage", bufs=4))
    zpool = ctx.enter_context(tc.tile_pool(name="zpool", bufs=2))
    psc = ctx.enter_context(tc.tile_pool(name="psc", bufs=2, space="PSUM"))
    psa = ctx.enter_context(tc.tile_pool(name="psa", bufs=1, space="PSUM"))
    pso = ctx.enter_context(tc.tile_pool(name="pso", bufs=1, space="PSUM"))

    # ---- resident input tiles ----
    qTs = const.tile([128, NET, L], BF16, tag="qTs")
    kTs = const.tile([128, NET, L], BF16, tag="kTs")
    vTs = const.tile([128, NET, L], BF16, tag="vTs")
    wqs = const.tile([128, NET, ROWS], BF16, tag="wqs")
    wks = const.tile([128, NET, ROWS], BF16, tag="wks")
    wvs = const.tile([128, NET, ROWS], BF16, tag="wvs")
    wos = const.tile([128, EMB], BF16, tag="wos")  # [hd, j]
    q3 = qT.rearrange("(o p) l -> p o l", p=128)
    k3 = kT.rearrange("(o p) l -> p o l", p=128)
    v3 = vT.rearrange("(o p) l -> p o l", p=128)

    def chunk(eng, dst, src3, lc):
        eng.dma_start(dst[:, :, bass.ts(lc, LT)], src3[:, :, bass.ts(lc, LT)])

    # ---- input DMA issue, consumption order ----
    # Only dependency-free DRAM->SBUF loads go on the scalar/sync HWDGE
    # rings: a data-dependent dma_start would block the issuing ENGINE
    # (scalar runs exp; sync runs the z-chains) at its semaphore wait.
    identb = const.tile([128, 128], BF16, tag="identb")
    nc.scalar.dma_start(wqs[:], wqT[:])
    chunk(nc.scalar, qTs, q3, 0)
    nc.scalar.dma_start(wvs[:], wvT[:])
    nc.scalar.dma_start(identb[:], identT[:])
    for lc in range(NLT):
        chunk(nc.scalar, vTs, v3, lc)
    nc.scalar.dma_start(wos[:], woT[:])
    # sync HWDGE: K-projection criticals, then k chunks with q tail mixed in
    nc.sync.dma_start(wks[:], wkT[:])
    chunk(nc.sync, kTs, k3, 0)
    chunk(nc.sync, kTs, k3, 1)
    chunk(nc.sync, kTs, k3, 2)
    chunk(nc.sync, qTs, q3, 1)
    chunk(nc.sync, kTs, k3, 3)
    chunk(nc.sync, qTs, q3, 2)
    chunk(nc.sync, qTs, q3, 3)

    # gpsimd SWDGE: mask chunks (fp8 DRAM -> bf16 SBUF casting DMAs)
    mask_tiles = {}

    def mask_fetch(b, splits):
        lt, h = divmod(b, HPC)
        mc = maskp.tile([128, NMT, LT], BF16, tag="maskc", name=f"maskc_{lt}_{h}")
        mask_tiles[b] = mc
        a = 0
        for n in splits:
            nc.gpsimd.dma_start(
                mc[:, a : a + n, :], maskP[lt, h, :, a : a + n, :]
            )
            a += n

    mask_fetch(0, QUADS)          # per-quad gating for the first block
    mask_fetch(1, (8, 8))

    # ---- working tiles ----
    QTb = const.tile([128, L], BF16, tag="QTb")
    KTb = const.tile([128, L], BF16, tag="KTb")
    VTb = const.tile([128, L], BF16, tag="VTb")
    vaug = const.tile([128, HPC, NMT, VROW], BF16, tag="vaug")
    nc.vector.memset(vaug[:, :, :, HEAD_DIM : HEAD_DIM + 1], 1.0)
    nc.vector.memset(vaug[:, :, :, HEAD_DIM + 1 : VROW], 0.0)
    attnTb = const.tile([128, L], BF16, tag="attnTb")

    def qk_proj(dst, w, x, lc):
        ps = pso.tile([128, LT], F32, tag="pso", name="ps_proj")
        for et in range(NET):
            nc.tensor.matmul(
                ps[:],
                lhsT=w[:, et, :],
                rhs=x[:, et, bass.ts(lc, LT)],
                start=(et == 0),
                stop=(et == NET - 1),
            )
        nc.vector.tensor_copy(out=dst[:, bass.ts(lc, LT)], in_=ps[:])

    def vt_proj(lc):
        qk_proj(VTb, wvs, vTs, lc)
        for mi in range(LT // MT):
            mt = lc * (LT // MT) + mi
            # PE transpose VT[d2h, m-tile] -> [m, d2h]; reuses the idle
            # outproj PSUM bank (outproj only starts at block 2)
            trp = pso.tile([128, MT], BF16, tag="pso", name=f"tr_{mt}")
            nc.tensor.transpose(trp[:], VTb[:, bass.ts(mt, MT)], identb[:])
            for h in range(HPC):
                nc.vector.tensor_copy(
                    out=vaug[:, h, mt, 0:HEAD_DIM],
                    in_=trp[:, bass.ts(h, HEAD_DIM)],
                )

    # ---- deferred epilogue pieces, drip-fed into later quads ----
    zdram = nc.dram_tensor("zdram", [NLT, HPC, LT], F32).ap()
    zidram = nc.dram_tensor("zidram", [NLT, HPC, LT], BF16).ap()
    state = {}
    pending = []  # [ready_quad, fn]
    quad_no = [0]

    def pop_pending(budget=3, flush=False):
        while pending and budget > 0:
            if not flush and pending[0][0] > quad_no[0]:
                break
            pending.pop(0)[1]()
            budget -= 1

    def piece_zstore(lt, h):
        def go():
            nc.sync.dma_start(
                zdram[lt, h][None, :], state[lt, h, "zseg"][:]
            )
        return go

    def piece_zload(lt, h):
        def go():
            zsp = zpool.tile([8, LT // 8], F32, tag="zsp", name=f"zsp_{lt}_{h}")
            nc.sync.dma_start(zsp[:], zdram[lt, h].rearrange("(o p) -> o p", o=8))
            state[lt, h, "zsp"] = zsp
        return go

    def piece_recip(lt, h):
        def go():
            zsp = state[lt, h, "zsp"]
            nc.vector.reciprocal(zsp[:], zsp[:])
            zspb = zpool.tile([8, LT // 8], BF16, tag="zspb", name=f"zspb_{lt}_{h}")
            nc.vector.tensor_copy(out=zspb[:], in_=zsp[:])
            nc.sync.dma_start(
                zidram[lt, h].rearrange("(o p) -> o p", o=8), zspb[:]
            )
        return go

    def piece_zbcast(lt, h):
        def go():
            # full-height tile so the norm's operands share a base partition
            zinvb = zpool.tile(
                [128, LT], BF16, tag="zinvb", name=f"zinvb_{lt}_{h}"
            )
            nc.sync.dma_start(
                zinvb[bass.ts(h, HEAD_DIM), :],
                zidram[lt, h][None, :].to_broadcast((HEAD_DIM, LT)),
            )
            state[lt, h, "zinvb"] = zinvb
        return go

    def piece_norm(lt, h):
        def go():
            ls = bass.ts(lt, LT)
            hd = bass.ts(h, HEAD_DIM)
            nc.vector.tensor_mul(
                out=attnTb[hd, ls],
                in0=attnTb[hd, ls],
                in1=state[lt, h, "zinvb"][hd, :],
            )
        return go

    def piece_outproj(lt, jt):
        def go():
            ls = bass.ts(lt, LT)
            ps = pso.tile([128, LT], F32, tag="pso", name="ps_out")
            nc.tensor.matmul(
                ps[:],
                lhsT=wos[:, bass.ts(jt, JT)],
                rhs=attnTb[:, ls],
                start=True,
                stop=True,
            )
            st = stage.tile([128, LT], F16, tag="st", name="st")
            nc.vector.tensor_copy(out=st[:], in_=ps[:])
            nc.gpsimd.dma_start(outT[bass.ts(jt, JT), ls], st[:])
        return go

    qk_proj(QTb, wqs, qTs, 0)

    # ---- attention blocks ----
    for b in range(NBLK):
        lt, h = divmod(b, HPC)
        ls = bass.ts(lt, LT)
        hd = bass.ts(h, HEAD_DIM)
        if b + 2 < NBLK:
            mask_fetch(b + 2, (NMT,))
        maskc = mask_tiles[b]
        pa = psa.tile([128, LT], F32, tag="psa", name=f"psa_{lt}_{h}")
        mt0 = 0
        prev_attn = None
        chunks_done = [0] if b == 0 else [NLT]
        for qi, qn in enumerate(QUADS):
            if b == 0:
                # interleave K/V projection chunks in consumption order
                need = min(NLT, (mt0 + qn + 3) // (LT // MT))
                while chunks_done[0] < need:
                    c = chunks_done[0]
                    qk_proj(KTb, wks, kTs, c)
                    vt_proj(c)
                    chunks_done[0] += 1
            if b == 1 and 1 <= qi <= 3:
                qk_proj(QTb, wqs, qTs, qi)  # PE filler + needed later
            pop_pending()
            ss = psc.tile([128, QB, LT], F32, tag="psc", name="ss")
            for i in range(qn):
                nc.tensor.matmul(
                    ss[:, i, :],
                    lhsT=KTb[hd, bass.ts(mt0 + i, MT)],
                    rhs=QTb[hd, ls],
                    start=True,
                    stop=True,
                )
            # one-quad software pipeline on PE: the previous quad's attn
            # matmuls are emitted AFTER this quad's scores, so the in-order
            # PE queue never blocks scores behind exp->mask-mult
            if prev_attn is not None:
                prev_attn()
            pT = ppool.tile([128, QB, LT], BF16, tag="pT", name="pT")
            nc.scalar.activation(
                pT[:, :qn, :], ss[:, :qn, :], mybir.ActivationFunctionType.Exp
            )
            nc.vector.tensor_mul(
                out=pT[:, :qn, :],
                in0=pT[:, :qn, :],
                in1=maskc[:, mt0 : mt0 + qn, :],
            )

            def make_attn(mt0=mt0, qn=qn, pT=pT, pa=pa, h=h):
                def go():
                    for i in range(qn):
                        mt = mt0 + i
                        nc.tensor.matmul(
                            pa[:VROW, :],
                            lhsT=vaug[:, h, mt, :],
                            rhs=pT[:, i, :],
                            start=(mt == 0),
                            stop=(mt == NMT - 1),
                        )
                return go

            prev_attn = make_attn()
            mt0 += qn
            quad_no[0] += 1
        prev_attn()
        nc.vector.tensor_copy(out=attnTb[hd, ls], in_=pa[0:HEAD_DIM, :])
        zseg = zpool.tile([1, LT], F32, tag="zseg", name=f"zseg_{lt}_{h}")
        nc.vector.tensor_copy(
            out=zseg[:], in_=pa[HEAD_DIM : HEAD_DIM + 1, :]
        )
        state[lt, h, "zseg"] = zseg
        q0 = quad_no[0]
        pending.append([q0 + 0, piece_zstore(lt, h)])
        pending.append([q0 + 1, piece_zload(lt, h)])
        pending.append([q0 + 2, piece_recip(lt, h)])
        pending.append([q0 + 3, piece_zbcast(lt, h)])
        pending.append([q0 + 4, piece_norm(lt, h)])
        if h == 1:
            for jt in range(NJT):
                pending.append([q0 + 4 + (jt + 1) // 2, piece_outproj(lt, jt)])

    pop_pending(budget=len(pending), flush=True)


_CACHE = {}


def _build():
    if "nc" in _CACHE:
        return _CACHE["nc"]
    nc = bacc.Bacc("TRN2", target_bir_lowering=False, debug=False,
                   num_devices=NCORES)
    qT = nc.dram_tensor("qT", [EMB, L], BF16, kind="ExternalInput").ap()
    kT = nc.dram_tensor("kT", [EMB, L], BF16, kind="ExternalInput").ap()
    vT = nc.dram_tensor("vT", [EMB, L], BF16, kind="ExternalInput").ap()
    wqT = nc.dram_tensor("wqT", [128, NET, ROWS], BF16, kind="ExternalInput").ap()
    wkT = nc.dram_tensor("wkT", [128, NET, ROWS], BF16, kind="ExternalInput").ap()
    wvT = nc.dram_tensor("wvT", [128, NET, ROWS], BF16, kind="ExternalInput").ap()
    woT = nc.dram_tensor("woT", [ROWS, EMB], BF16, kind="ExternalInput").ap()
    maskP = nc.dram_tensor(
        "maskP", [NLT, HPC, 128, NMT, LT], FP8, kind="ExternalInput"
    ).ap()
    identT = nc.dram_tensor("identT", [128, 128], BF16, kind="ExternalInput").ap()
    outT = nc.dram_tensor("outT", [EMB, L], F16, kind="ExternalOutput").ap()

    with tile.TileContext(nc) as tc:
        _mha_kernel(tc, outT, qT, kT, vT, wqT, wkT, wvT, woT, maskP, identT)
    nc.compile()
    _CACHE["nc"] = nc
    return nc


def _pack_w(w):
    # [ROWS, EMB] -> w.T [EMB, ROWS] -> [128, NET, ROWS] with e = o*128+p
    return np.ascontiguousarray(
        w.T.reshape(NET, 128, ROWS).transpose(1, 0, 2)
    ).astype(NPBF16)


def _pack_mask(keep):
    # keep [HPC, l(query), m(key)] -> keepT [HPC, m, l]
    # -> [NLT, HPC, 128(p), NMT(mo), LT] chunk-contiguous, m = mo*128+p
    keepT = keep.swapaxes(1, 2)
    m5 = keepT.reshape(HPC, NMT, 128, NLT, LT).transpose(3, 0, 2, 1, 4)
    return np.ascontiguousarray(m5).astype(NPFP8)


def _prep_in_maps(q, k, v, mask, Wq, Wk, Wv, Wo):
    qT = np.ascontiguousarray(q.T).astype(NPBF16)
    kT = np.ascontiguousarray(k.T).astype(NPBF16)
    vT = np.ascontiguousarray(v.T).astype(NPBF16)
    in_maps = []
    for c in range(NCORES):
        rows = slice(c * ROWS, (c + 1) * ROWS)
        in_maps.append({
            "qT": qT,
            "kT": kT,
            "vT": vT,
            "wqT": _pack_w(Wq[rows] * SCALE),
            "wkT": _pack_w(Wk[rows]),
            "wvT": _pack_w(Wv[rows]),
            "woT": np.ascontiguousarray(Wo[:, rows].T).astype(NPBF16),
            "maskP": _pack_mask(~mask[c * HPC : (c + 1) * HPC]),
            "identT": np.eye(128, dtype=NPBF16),
        })
    return in_maps


def run(q, k, v, mask, Wq, Wk, Wv, Wo, **spmd_kwargs):
    nc = _build()
    in_maps = _prep_in_maps(q, k, v, mask, Wq, Wk, Wv, Wo)
    res = run_bass_kernel_spmd(nc, in_maps, list(range(NCORES)), **spmd_kwargs)
    outT = np.zeros((EMB, L), np.float64)
    for r in res.results:
        outT += r["outT"].astype(np.float64)
    out = np.ascontiguousarray(outT.T).astype(np.float32)
    return out, res


def kernel(q, k, v, mask, Wq, Wk, Wv, Wo):
    q, k, v = (np.asarray(x, np.float32) for x in (q, k, v))
    Wq, Wk, Wv, Wo = (np.asarray(x, np.float32) for x in (Wq, Wk, Wv, Wo))
    mask = np.asarray(mask, bool)
    out, _ = run(q, k, v, mask, Wq, Wk, Wv, Wo)
    return out


# revision 19
# speedup vs baseline: 1.0468x; 1.0018x over previous
"""Multi-head attention (L=2048, EMB=1024, H=16, D=64) on 8 TRN2 NeuronCores.

Tensor-parallel over heads: core i owns heads {2i, 2i+1} (a 128-row block of
Wq/Wk/Wv and a 128-column block of Wo). Each core computes its two heads'
attention plus its partial output projection; the host sums the 8 partials.

Device-side layout is fully transposed (scores^T = [m, l]) so no on-device
transposes of the score matrix are needed:
  QT[d, l] = (Wq_shard @ q^T)        lhsT = (Wq_shard/8)^T, rhs = q^T
  KT[d, l] = (Wk_shard @ k^T)
  VT[d, m] = (Wv_shard @ v^T)        then XBAR DMA-transpose -> vaug[m, d]
  sT[m, l] = KT_h^T @ QT_h           (per head, contraction d=64)
  pT       = exp(sT) * keepT         (no max-subtraction: |s| <~ 9)
  attnT/Z  = [V_h | 1]^T @ pT        (ones column gives softmax denominator)
  outT     = Wo_shard^T-block @ (attnT / Z)   partial, summed on host

Key optimizations over the straightforward version (from NTFF traces):
- mask ships as fp8 (0.0/1.0) in a chunk-contiguous DRAM layout and is
  cast to bf16 in-flight by the gpsimd SWDGE ring: halves mask HBM reads
  and makes every DMA packet dense.
- output partials ship as f16 (halves output traffic; ~0.05% noise).
- V projection computes VT with 32 big matmuls + 16 XBAR DMA transposes
  instead of 128 tiny matmuls (PE time for V drops ~4x).
- input DMAs are issued in exact consumption order across the three
  rings (scalar/sync HWDGE + gpsimd SWDGE) so the first score matmul
  isn't stuck behind bulk traffic.
- per-(lt,h) softmax-denominator chains (DRAM bounce to spread the
  reciprocal over 8 partitions) and the output projection are deferred
  and drip-fed into the following block's quad stream; the final
  block's epilogue is flushed ASAP to keep the tail short.
- PSUM: 6 banks double-buffered scores, 1 bank attention accumulator,
  1 bank shared by projections + output projection.
"""

import sys

for _p in ("/opt/trn_rl_repo",):
    if _p not in sys.path:
        sys.path.insert(0, _p)

from contextlib import ExitStack

import ml_dtypes
import numpy as np

import concourse.bass as bass
import concourse.tile as tile
from concourse import bacc, mybir
from concourse._compat import with_exitstack
from concourse.bass_utils import run_bass_kernel_spmd

BF16 = mybir.dt.bfloat16
FP8 = mybir.dt.float8e4
F16 = mybir.dt.float16
F32 = mybir.dt.float32
NPBF16 = ml_dtypes.bfloat16
NPFP8 = ml_dtypes.float8_e4m3
NPF16 = np.float16

L = 2048
EMB = 1024
NHEAD = 16
HEAD_DIM = 64
NCORES = 8
HPC = NHEAD // NCORES  # heads per core = 2
ROWS = HPC * HEAD_DIM  # weight rows per core = 128
SCALE = HEAD_DIM ** -0.5

LT = 512               # l-tile (matmul free dim / PSUM bank)
NLT = L // LT          # 4
MT = 128               # m-tile (key-block on partitions)
NMT = L // MT          # 16
ET = 128               # contraction tile over EMB
NET = EMB // ET        # 8
JT = 128               # output-row tile
NJT = EMB // JT        # 8
NBLK = NLT * HPC       # 8 (lt, h) blocks

QUADS = (3, 3, 3, 3, 2, 2)   # m-tiles per exp/mask-mult instruction
QB = 3                        # psc tile m-capacity (PSUM banks per slot)
VROW = 66                     # 64 V cols + ones + pad


@with_exitstack
def _mha_kernel(ctx, tc, outT, qT, kT, vT, wqT, wkT, wvT, woT, maskP, identT):
    nc = tc.nc

    const = ctx.enter_context(tc.tile_pool(name="const", bufs=1))
    ppool = ctx.enter_context(tc.tile_pool(name="ptiles", bufs=4))
    maskp = ctx.enter_context(tc.tile_pool(name="maskp", bufs=3))
    stage = ctx.enter_context(tc.tile_pool(name="stage", bufs=4))
    zpool = ctx.enter_context(tc.tile_pool(name="zpool", bufs=2))
    psc = ctx.enter_context(tc.tile_pool(name="psc", bufs=2, space="PSUM"))
    psa = ctx.enter_context(tc.tile_pool(name="psa", bufs=1, space="PSUM"))
    pso = ctx.enter_context(tc.tile_pool(name="pso", bufs=1, space="PSUM"))

    # ---- resident input tiles ----
    qTs = const.tile([128, NET, L], BF16, tag="qTs")
    kTs = const.tile([128, NET, L], BF16, tag="kTs")
    vTs = const.tile([128, NET, L], BF16, tag="vTs")
    wqs = const.tile([128, NET, ROWS], BF16, tag="wqs")
    wks = const.tile([128, NET, ROWS], BF16, tag="wks")
    wvs = const.tile([128, NET, ROWS], BF16, tag="wvs")
    wos = const.tile([128, EMB], BF16, tag="wos")  # [hd, j]
    # inputs arrive chunk-contiguous ([NLT, 128, NET, LT] in DRAM) so every
    # DMA packet is a dense 2KB run
    def chunk(eng, dst, src4, lc):
        eng.dma_start(dst[:, :, bass.ts(lc, LT)], src4[lc])

    # ---- input DMA issue, consumption order ----
    # Only dependency-free DRAM->SBUF loads go on the scalar/sync HWDGE
    # rings: a data-dependent dma_start would block the issuing ENGINE
    # (scalar runs exp; sync runs the z-chains) at its semaphore wait.
    identb = const.tile([128, 128], BF16, tag="identb")
    nc.scalar.dma_start(wqs[:], wqT[:])
    chunk(nc.scalar, qTs, qT, 0)
    nc.scalar.dma_start(wvs[:], wvT[:])
    nc.scalar.dma_start(identb[:], identT[:])
    for lc in range(NLT):
        chunk(nc.scalar, vTs, vT, lc)
    nc.scalar.dma_start(wos[:], woT[:])
    # sync HWDGE: K-projection criticals, then k chunks with q tail mixed in
    nc.sync.dma_start(wks[:], wkT[:])
    chunk(nc.sync, kTs, kT, 0)
    chunk(nc.sync, kTs, kT, 1)
    chunk(nc.sync, kTs, kT, 2)
    chunk(nc.sync, qTs, qT, 1)
    chunk(nc.sync, kTs, kT, 3)
    chunk(nc.sync, qTs, qT, 2)
    chunk(nc.sync, qTs, qT, 3)

    # gpsimd SWDGE: mask chunks (fp8 DRAM -> bf16 SBUF casting DMAs)
    mask_tiles = {}

    def mask_fetch(b, splits):
        lt, h = divmod(b, HPC)
        mc = maskp.tile([128, NMT, LT], BF16, tag="maskc", name=f"maskc_{lt}_{h}")
        mask_tiles[b] = mc
        a = 0
        for n in splits:
            nc.gpsimd.dma_start(
                mc[:, a : a + n, :], maskP[lt, h, :, a : a + n, :]
            )
            a += n

    mask_fetch(0, QUADS)          # per-quad gating for the first block
    # later chunks are issued mid-block so they don't steal startup bandwidth

    # ---- working tiles ----
    QTb = const.tile([128, L], BF16, tag="QTb")
    KTb = const.tile([128, L], BF16, tag="KTb")
    VTb = const.tile([128, L], BF16, tag="VTb")
    vaug = const.tile([128, HPC, NMT, VROW], BF16, tag="vaug")
    nc.vector.memset(vaug[:, :, :, HEAD_DIM : HEAD_DIM + 1], 1.0)
    nc.vector.memset(vaug[:, :, :, HEAD_DIM + 1 : VROW], 0.0)
    attnTb = const.tile([128, L], BF16, tag="attnTb")
    onesb = const.tile([1, 128], BF16, tag="onesb")
    nc.vector.memset(onesb[:], 1.0)

    def qk_proj(dst, w, x, lc):
        ps = pso.tile([128, LT], F32, tag="pso", name="ps_proj")
        for et in range(NET):
            nc.tensor.matmul(
                ps[:],
                lhsT=w[:, et, :],
                rhs=x[:, et, bass.ts(lc, LT)],
                start=(et == 0),
                stop=(et == NET - 1),
            )
        nc.vector.tensor_copy(out=dst[:, bass.ts(lc, LT)], in_=ps[:])

    def vt_proj(lc):
        qk_proj(VTb, wvs, vTs, lc)
        for mi in range(LT // MT):
            mt = lc * (LT // MT) + mi
            # PE transpose VT[d2h, m-tile] -> [m, d2h]; reuses the idle
            # outproj PSUM bank (outproj only starts at block 2)
            trp = pso.tile([128, MT], BF16, tag="pso", name=f"tr_{mt}")
            nc.tensor.transpose(trp[:], VTb[:, bass.ts(mt, MT)], identb[:])
            for h in range(HPC):
                nc.vector.tensor_copy(
                    out=vaug[:, h, mt, 0:HEAD_DIM],
                    in_=trp[:, bass.ts(h, HEAD_DIM)],
                )

    # ---- deferred epilogue pieces, drip-fed into later quads ----
    zdram = nc.dram_tensor("zdram", [NLT, HPC, LT], F32).ap()
    zidram = nc.dram_tensor("zidram", [NLT, HPC, LT], BF16).ap()
    state = {}
    pending = []  # [ready_quad, fn]
    quad_no = [0]

    def pop_pending(budget=3, flush=False):
        while pending and budget > 0:
            if not flush and pending[0][0] > quad_no[0]:
                break
            pending.pop(0)[1]()
            budget -= 1

    def piece_zstore(lt, h):
        def go():
            nc.sync.dma_start(
                zdram[lt, h][None, :], state[lt, h, "zseg"][:]
            )
        return go

    def piece_zload(lt, h):
        def go():
            zsp = zpool.tile([8, LT // 8], F32, tag="zsp", name=f"zsp_{lt}_{h}")
            nc.sync.dma_start(zsp[:], zdram[lt, h].rearrange("(o p) -> o p", o=8))
            state[lt, h, "zsp"] = zsp
        return go

    def piece_recip(lt, h):
        def go():
            zsp = state[lt, h, "zsp"]
            nc.vector.reciprocal(zsp[:], zsp[:])
            zspb = zpool.tile([8, LT // 8], BF16, tag="zspb", name=f"zspb_{lt}_{h}")
            nc.vector.tensor_copy(out=zspb[:], in_=zsp[:])
            nc.sync.dma_start(
                zidram[lt, h].rearrange("(o p) -> o p", o=8), zspb[:]
            )
        return go

    def piece_zbcast(lt, h):
        def go():
            # full-height tile so the norm's operands share a base partition
            zinvb = zpool.tile(
                [128, LT], BF16, tag="zinvb", name=f"zinvb_{lt}_{h}"
            )
            nc.sync.dma_start(
                zinvb[bass.ts(h, HEAD_DIM), :],
                zidram[lt, h][None, :].to_broadcast((HEAD_DIM, LT)),
            )
            state[lt, h, "zinvb"] = zinvb
        return go

    def piece_norm(lt, h):
        def go():
            ls = bass.ts(lt, LT)
            hd = bass.ts(h, HEAD_DIM)
            nc.vector.tensor_mul(
                out=attnTb[hd, ls],
                in0=attnTb[hd, ls],
                in1=state[lt, h, "zinvb"][hd, :],
            )
        return go

    def piece_recip_short(lt, h):
        # last-l-tile fast path: single-partition reciprocal, no DRAM bounce
        def go():
            zseg = state[lt, h, "zseg"]
            nc.vector.reciprocal(zseg[:], zseg[:])
            zsegb = zpool.tile([1, LT], BF16, tag="zsegb", name=f"zsegb_{lt}_{h}")
            nc.vector.tensor_copy(out=zsegb[:], in_=zseg[:])
            state[lt, h, "zsegb"] = zsegb
        return go

    def piece_bcast_pe(lt, h):
        # broadcast zinv across partitions with a ones-column matmul
        def go():
            zb = pso.tile([128, LT], F32, tag="pso", name=f"zb_{lt}_{h}")
            nc.tensor.matmul(
                zb[:], lhsT=onesb[:], rhs=state[lt, h, "zsegb"][:],
                start=True, stop=True,
            )
            state[lt, h, "zinvps"] = zb
        return go

    def piece_norm_short(lt, h):
        def go():
            ls = bass.ts(lt, LT)
            hd = bass.ts(h, HEAD_DIM)
            nc.vector.tensor_mul(
                out=attnTb[hd, ls],
                in0=attnTb[hd, ls],
                in1=state[lt, h, "zinvps"][hd, :],
            )
        return go

    def piece_outproj(lt, jt):
        def go():
            ls = bass.ts(lt, LT)
            ps = pso.tile([128, LT], F32, tag="pso", name="ps_out")
            nc.tensor.matmul(
                ps[:],
                lhsT=wos[:, bass.ts(jt, JT)],
                rhs=attnTb[:, ls],
                start=True,
                stop=True,
            )
            st = stage.tile([128, LT], F16, tag="st", name="st")
            nc.vector.tensor_copy(out=st[:], in_=ps[:])
            nc.gpsimd.dma_start(outT[bass.ts(jt, JT), ls], st[:])
        return go

    qk_proj(QTb, wqs, qTs, 0)

    # ---- attention blocks ----
    for b in range(NBLK):
        lt, h = divmod(b, HPC)
        ls = bass.ts(lt, LT)
        hd = bass.ts(h, HEAD_DIM)
        maskc = mask_tiles[b]
        pa = psa.tile([128, LT], F32, tag="psa", name=f"psa_{lt}_{h}")
        mt0 = 0
        prev_attn = None
        chunks_done = [0] if b == 0 else [NLT]
        for qi, qn in enumerate(QUADS):
            if b == 0:
                # interleave K/V projection chunks in consumption order
                need = min(NLT, (mt0 + qn + 3) // (LT // MT))
                while chunks_done[0] < need:
                    c = chunks_done[0]
                    qk_proj(KTb, wks, kTs, c)
                    vt_proj(c)
                    chunks_done[0] += 1
            if b == 1 and 1 <= qi <= 3:
                qk_proj(QTb, wqs, qTs, qi)  # PE filler + needed later
            if b == 0 and qi == 2:
                mask_fetch(1, (8, 8))
            if qi == 3 and b + 2 < NBLK:
                mask_fetch(b + 2, (NMT,))
            pop_pending()
            ss = psc.tile([128, QB, LT], F32, tag="psc", name="ss")
            for i in range(qn):
                nc.tensor.matmul(
                    ss[:, i, :],
                    lhsT=KTb[hd, bass.ts(mt0 + i, MT)],
                    rhs=QTb[hd, ls],
                    start=True,
                    stop=True,
                )
            # one-quad software pipeline on PE: the previous quad's attn
            # matmuls are emitted AFTER this quad's scores, so the in-order
            # PE queue never blocks scores behind exp->mask-mult
            if prev_attn is not None:
                prev_attn()
            pT = ppool.tile([128, QB, LT], BF16, tag="pT", name="pT")
            nc.scalar.activation(
                pT[:, :qn, :], ss[:, :qn, :], mybir.ActivationFunctionType.Exp
            )
            nc.vector.tensor_mul(
                out=pT[:, :qn, :],
                in0=pT[:, :qn, :],
                in1=maskc[:, mt0 : mt0 + qn, :],
            )

            def make_attn(mt0=mt0, qn=qn, pT=pT, pa=pa, h=h):
                def go():
                    for i in range(qn):
                        mt = mt0 + i
                        nc.tensor.matmul(
                            pa[:VROW, :],
                            lhsT=vaug[:, h, mt, :],
                            rhs=pT[:, i, :],
                            start=(mt == 0),
                            stop=(mt == NMT - 1),
                        )
                return go

            prev_attn = make_attn()
            mt0 += qn
            quad_no[0] += 1
        prev_attn()
        nc.vector.tensor_copy(out=attnTb[hd, ls], in_=pa[0:HEAD_DIM, :])
        zseg = zpool.tile([1, LT], F32, tag="zseg", name=f"zseg_{lt}_{h}")
        nc.vector.tensor_copy(
            out=zseg[:], in_=pa[HEAD_DIM : HEAD_DIM + 1, :]
        )
        state[lt, h, "zseg"] = zseg
        q0 = quad_no[0]
        if lt == NLT - 1:
            # short DMA-free chain so the final epilogue isn't
            # serialized behind 4 DMA-latency hops
            pending.append([q0 + 0, piece_recip_short(lt, h)])
            pending.append([q0 + 1, piece_bcast_pe(lt, h)])
            pending.append([q0 + 2, piece_norm_short(lt, h)])
        else:
            pending.append([q0 + 0, piece_zstore(lt, h)])
            pending.append([q0 + 1, piece_zload(lt, h)])
            pending.append([q0 + 2, piece_recip(lt, h)])
            pending.append([q0 + 3, piece_zbcast(lt, h)])
            pending.append([q0 + 4, piece_norm(lt, h)])
        if h == 1:
            for jt in range(NJT):
                pending.append([q0 + 4 + (jt + 1) // 2, piece_outproj(lt, jt)])

    pop_pending(budget=len(pending), flush=True)


_CACHE = {}


def _build():
    if "nc" in _CACHE:
        return _CACHE["nc"]
    nc = bacc.Bacc("TRN2", target_bir_lowering=False, debug=False,
                   num_devices=NCORES)
    qT = nc.dram_tensor("qT", [NLT, 128, NET, LT], BF16, kind="ExternalInput").ap()
    kT = nc.dram_tensor("kT", [NLT, 128, NET, LT], BF16, kind="ExternalInput").ap()
    vT = nc.dram_tensor("vT", [NLT, 128, NET, LT], BF16, kind="ExternalInput").ap()
    wqT = nc.dram_tensor("wqT", [128, NET, ROWS], BF16, kind="ExternalInput").ap()
    wkT = nc.dram_tensor("wkT", [128, NET, ROWS], BF16, kind="ExternalInput").ap()
    wvT = nc.dram_tensor("wvT", [128, NET, ROWS], BF16, kind="ExternalInput").ap()
    woT = nc.dram_tensor("woT", [ROWS, EMB], BF16, kind="ExternalInput").ap()
    maskP = nc.dram_tensor(
        "maskP", [NLT, HPC, 128, NMT, LT], FP8, kind="ExternalInput"
    ).ap()
    identT = nc.dram_tensor("identT", [128, 128], BF16, kind="ExternalInput").ap()
    outT = nc.dram_tensor("outT", [EMB, L], F16, kind="ExternalOutput").ap()

    with tile.TileContext(nc) as tc:
        _mha_kernel(tc, outT, qT, kT, vT, wqT, wkT, wvT, woT, maskP, identT)
    nc.compile()
    _CACHE["nc"] = nc
    return nc


def _pack_w(w):
    # [ROWS, EMB] -> w.T [EMB, ROWS] -> [128, NET, ROWS] with e = o*128+p
    return np.ascontiguousarray(
        w.T.reshape(NET, 128, ROWS).transpose(1, 0, 2)
    ).astype(NPBF16)


def _pack_mask(keep):
    # keep [HPC, l(query), m(key)] -> keepT [HPC, m, l]
    # -> [NLT, HPC, 128(p), NMT(mo), LT] chunk-contiguous, m = mo*128+p
    keepT = keep.swapaxes(1, 2)
    m5 = keepT.reshape(HPC, NMT, 128, NLT, LT).transpose(3, 0, 2, 1, 4)
    return np.ascontiguousarray(m5).astype(NPFP8)


def _pack_x(x):
    # [L, EMB] -> x.T [EMB, L] -> [NLT, 128(p), NET(o), LT] chunk-contiguous
    # (e = o*128+p) so every partition row of a chunk is one 8KB dense run
    return np.ascontiguousarray(
        x.T.reshape(NET, 128, NLT, LT).transpose(2, 1, 0, 3)
    ).astype(NPBF16)


def _prep_in_maps(q, k, v, mask, Wq, Wk, Wv, Wo):
    qT = _pack_x(q)
    kT = _pack_x(k)
    vT = _pack_x(v)
    in_maps = []
    for c in range(NCORES):
        rows = slice(c * ROWS, (c + 1) * ROWS)
        in_maps.append({
            "qT": qT,
            "kT": kT,
            "vT": vT,
            "wqT": _pack_w(Wq[rows] * SCALE),
            "wkT": _pack_w(Wk[rows]),
            "wvT": _pack_w(Wv[rows]),
            "woT": np.ascontiguousarray(Wo[:, rows].T).astype(NPBF16),
            "maskP": _pack_mask(~mask[c * HPC : (c + 1) * HPC]),
            "identT": np.eye(128, dtype=NPBF16),
        })
    return in_maps


def run(q, k, v, mask, Wq, Wk, Wv, Wo, **spmd_kwargs):
    nc = _build()
    in_maps = _prep_in_maps(q, k, v, mask, Wq, Wk, Wv, Wo)
    res = run_bass_kernel_spmd(nc, in_maps, list(range(NCORES)), **spmd_kwargs)
    outT = np.zeros((EMB, L), np.float64)
    for r in res.results:
        outT += r["outT"].astype(np.float64)
    out = np.ascontiguousarray(outT.T).astype(np.float32)
    return out, res


def kernel(q, k, v, mask, Wq, Wk, Wv, Wo):
    q, k, v = (np.asarray(x, np.float32) for x in (q, k, v))
    Wq, Wk, Wv, Wo = (np.asarray(x, np.float32) for x in (Wq, Wk, Wv, Wo))
    mask = np.asarray(mask, bool)
    out, _ = run(q, k, v, mask, Wq, Wk, Wv, Wo)
    return out


# revision 23
# speedup vs baseline: 1.0476x; 1.0008x over previous
"""Multi-head attention (L=2048, EMB=1024, H=16, D=64) on 8 TRN2 NeuronCores.

Tensor-parallel over heads: core i owns heads {2i, 2i+1} (a 128-row block of
Wq/Wk/Wv and a 128-column block of Wo). Each core computes its two heads'
attention plus its partial output projection; the host sums the 8 partials.

Device-side layout is fully transposed (scores^T = [m, l]) so no on-device
transposes of the score matrix are needed:
  QT[d, l] = (Wq_shard @ q^T)        lhsT = (Wq_shard/8)^T, rhs = q^T
  KT[d, l] = (Wk_shard @ k^T)
  VT[d, m] = (Wv_shard @ v^T)        then XBAR DMA-transpose -> vaug[m, d]
  sT[m, l] = KT_h^T @ QT_h           (per head, contraction d=64)
  pT       = exp(sT) * keepT         (no max-subtraction: |s| <~ 9)
  attnT/Z  = [V_h | 1]^T @ pT        (ones column gives softmax denominator)
  outT     = Wo_shard^T-block @ (attnT / Z)   partial, summed on host

Key optimizations over the straightforward version (from NTFF traces):
- mask ships as fp8 (0.0/1.0) in a chunk-contiguous DRAM layout and is
  cast to bf16 in-flight by the gpsimd SWDGE ring: halves mask HBM reads
  and makes every DMA packet dense.
- output partials ship as f16 (halves output traffic; ~0.05% noise).
- V projection computes VT with 32 big matmuls + 16 XBAR DMA transposes
  instead of 128 tiny matmuls (PE time for V drops ~4x).
- input DMAs are issued in exact consumption order across the three
  rings (scalar/sync HWDGE + gpsimd SWDGE) so the first score matmul
  isn't stuck behind bulk traffic.
- per-(lt,h) softmax-denominator chains (DRAM bounce to spread the
  reciprocal over 8 partitions) and the output projection are deferred
  and drip-fed into the following block's quad stream; the final
  block's epilogue is flushed ASAP to keep the tail short.
- PSUM: 6 banks double-buffered scores, 1 bank attention accumulator,
  1 bank shared by projections + output projection.
"""

import sys

for _p in ("/opt/trn_rl_repo",):
    if _p not in sys.path:
        sys.path.insert(0, _p)

from contextlib import ExitStack

import ml_dtypes
import numpy as np

import concourse.bass as bass
import concourse.tile as tile
from concourse import bacc, mybir
from concourse._compat import with_exitstack
from concourse.bass_utils import run_bass_kernel_spmd

BF16 = mybir.dt.bfloat16
FP8 = mybir.dt.float8e4
F16 = mybir.dt.float16
F32 = mybir.dt.float32
NPBF16 = ml_dtypes.bfloat16
NPFP8 = ml_dtypes.float8_e4m3
NPF16 = np.float16

L = 2048
EMB = 1024
NHEAD = 16
HEAD_DIM = 64
NCORES = 8
HPC = NHEAD // NCORES  # heads per core = 2
ROWS = HPC * HEAD_DIM  # weight rows per core = 128
SCALE = HEAD_DIM ** -0.5

LT = 512               # l-tile (matmul free dim / PSUM bank)
NLT = L // LT          # 4
MT = 128               # m-tile (key-block on partitions)
NMT = L // MT          # 16
ET = 128               # contraction tile over EMB
NET = EMB // ET        # 8
JT = 128               # output-row tile
NJT = EMB // JT        # 8
NBLK = NLT * HPC       # 8 (lt, h) blocks

QUADS = (3, 3, 3, 3, 2, 2)   # m-tiles per exp/mask-mult instruction
QB = 3                        # psc tile m-capacity (PSUM banks per slot)
VROW = 66                     # 64 V cols + ones + pad


@with_exitstack
def _mha_kernel(ctx, tc, outT, qT, kT, vT, wqT, wkT, wvT, woT, maskP, identT):
    nc = tc.nc

    const = ctx.enter_context(tc.tile_pool(name="const", bufs=1))
    ppool = ctx.enter_context(tc.tile_pool(name="ptiles", bufs=4))
    maskp = ctx.enter_context(tc.tile_pool(name="maskp", bufs=3))
    stage = ctx.enter_context(tc.tile_pool(name="stage", bufs=4))
    zpool = ctx.enter_context(tc.tile_pool(name="zpool", bufs=2))
    psc = ctx.enter_context(tc.tile_pool(name="psc", bufs=2, space="PSUM"))
    psa = ctx.enter_context(tc.tile_pool(name="psa", bufs=1, space="PSUM"))
    pso = ctx.enter_context(tc.tile_pool(name="pso", bufs=1, space="PSUM"))

    # ---- resident input tiles ----
    qTs = const.tile([128, NET, L], BF16, tag="qTs")
    kTs = const.tile([128, NET, L], BF16, tag="kTs")
    vTs = const.tile([128, NET, L], BF16, tag="vTs")
    wqs = const.tile([128, NET, ROWS], BF16, tag="wqs")
    wks = const.tile([128, NET, ROWS], BF16, tag="wks")
    wvs = const.tile([128, NET, ROWS], BF16, tag="wvs")
    wos = const.tile([128, EMB], BF16, tag="wos")  # [hd, j]
    # inputs arrive chunk-contiguous ([NLT, 128, NET, LT] in DRAM) so every
    # DMA packet is a dense 2KB run
    def chunk(eng, dst, src4, lc):
        eng.dma_start(dst[:, :, bass.ts(lc, LT)], src4[lc])

    # ---- input DMA issue, consumption order ----
    # Only dependency-free DRAM->SBUF loads go on the scalar/sync HWDGE
    # rings: a data-dependent dma_start would block the issuing ENGINE
    # (scalar runs exp; sync runs the z-chains) at its semaphore wait.
    # The issuing ENGINE is occupied for the whole transfer of each DMA it
    # starts. So: scalar (runs exp) issues nothing; sync carries only the
    # critical first chunks and is clear before the z-chains start; gpsimd
    # carries the bulk tails + masks + output stores.
    identb = const.tile([128, 128], BF16, tag="identb")
    nc.sync.dma_start(wqs[:], wqT[:])
    chunk(nc.sync, qTs, qT, 0)
    nc.sync.dma_start(wks[:], wkT[:])
    chunk(nc.sync, kTs, kT, 0)
    nc.sync.dma_start(wvs[:], wvT[:])
    nc.sync.dma_start(identb[:], identT[:])
    chunk(nc.sync, vTs, vT, 0)
    chunk(nc.sync, kTs, kT, 1)
    chunk(nc.sync, vTs, vT, 1)

    # gpsimd SWDGE: mask chunks (fp8 DRAM -> bf16 SBUF casting DMAs)
    mask_tiles = {}

    def mask_fetch(b, splits):
        lt, h = divmod(b, HPC)
        mc = maskp.tile([128, NMT, LT], BF16, tag="maskc", name=f"maskc_{lt}_{h}")
        mask_tiles[b] = mc
        a = 0
        for n in splits:
            nc.gpsimd.dma_start(
                mc[:, a : a + n, :], maskP[lt, h, :, a : a + n, :]
            )
            a += n

    mask_fetch(0, QUADS)          # per-quad gating for the first block
    # k/v/q bulk tails ride the gpsimd ring behind the first mask chunk,
    # in consumption order; later masks are issued mid-block
    chunk(nc.gpsimd, kTs, kT, 2)
    chunk(nc.gpsimd, vTs, vT, 2)
    chunk(nc.gpsimd, kTs, kT, 3)
    chunk(nc.gpsimd, vTs, vT, 3)
    mask_fetch(1, (8, 8))
    chunk(nc.gpsimd, qTs, qT, 1)
    chunk(nc.gpsimd, qTs, qT, 2)
    chunk(nc.gpsimd, qTs, qT, 3)
    nc.gpsimd.dma_start(wos[:], woT[:])

    # ---- working tiles ----
    QTb = const.tile([128, L], BF16, tag="QTb")
    KTb = const.tile([128, L], BF16, tag="KTb")
    VTb = const.tile([128, L], BF16, tag="VTb")
    vaug = const.tile([128, HPC, NMT, VROW], BF16, tag="vaug")
    nc.vector.memset(vaug[:, :, :, HEAD_DIM : HEAD_DIM + 1], 1.0)
    nc.vector.memset(vaug[:, :, :, HEAD_DIM + 1 : VROW], 0.0)
    attnTb = const.tile([128, L], BF16, tag="attnTb")
    onesb = const.tile([1, 128], BF16, tag="onesb")
    nc.vector.memset(onesb[:], 1.0)

    def qk_proj(dst, w, x, lc):
        ps = pso.tile([128, LT], F32, tag="pso", name="ps_proj")
        for et in range(NET):
            nc.tensor.matmul(
                ps[:],
                lhsT=w[:, et, :],
                rhs=x[:, et, bass.ts(lc, LT)],
                start=(et == 0),
                stop=(et == NET - 1),
            )
        nc.vector.tensor_copy(out=dst[:, bass.ts(lc, LT)], in_=ps[:])

    def vt_proj(lc):
        qk_proj(VTb, wvs, vTs, lc)
        for mi in range(LT // MT):
            mt = lc * (LT // MT) + mi
            # PE transpose VT[d2h, m-tile] -> [m, d2h]; reuses the idle
            # outproj PSUM bank (outproj only starts at block 2)
            trp = pso.tile([128, MT], BF16, tag="pso", name=f"tr_{mt}")
            nc.tensor.transpose(trp[:], VTb[:, bass.ts(mt, MT)], identb[:])
            for h in range(HPC):
                nc.vector.tensor_copy(
                    out=vaug[:, h, mt, 0:HEAD_DIM],
                    in_=trp[:, bass.ts(h, HEAD_DIM)],
                )

    # ---- deferred epilogue pieces, drip-fed into later quads ----
    zdram = nc.dram_tensor("zdram", [NLT, HPC, LT], F32).ap()
    zidram = nc.dram_tensor("zidram", [NLT, HPC, LT], BF16).ap()
    state = {}
    pending = []  # [ready_quad, fn]
    quad_no = [0]

    def pop_pending(budget=3, flush=False):
        while pending and budget > 0:
            if not flush and pending[0][0] > quad_no[0]:
                break
            pending.pop(0)[1]()
            budget -= 1

    def piece_zstore(lt, h):
        def go():
            nc.sync.dma_start(
                zdram[lt, h][None, :], state[lt, h, "zseg"][:]
            )
        return go

    def piece_zload(lt, h):
        def go():
            zsp = zpool.tile([8, LT // 8], F32, tag="zsp", name=f"zsp_{lt}_{h}")
            nc.sync.dma_start(zsp[:], zdram[lt, h].rearrange("(o p) -> o p", o=8))
            state[lt, h, "zsp"] = zsp
        return go

    def piece_recip(lt, h):
        def go():
            zsp = state[lt, h, "zsp"]
            nc.vector.reciprocal(zsp[:], zsp[:])
            zspb = zpool.tile([8, LT // 8], BF16, tag="zspb", name=f"zspb_{lt}_{h}")
            nc.vector.tensor_copy(out=zspb[:], in_=zsp[:])
            nc.sync.dma_start(
                zidram[lt, h].rearrange("(o p) -> o p", o=8), zspb[:]
            )
        return go

    def piece_zbcast(lt, h):
        def go():
            # full-height tile so the norm's operands share a base partition
            zinvb = zpool.tile(
                [128, LT], BF16, tag="zinvb", name=f"zinvb_{lt}_{h}"
            )
            nc.sync.dma_start(
                zinvb[bass.ts(h, HEAD_DIM), :],
                zidram[lt, h][None, :].to_broadcast((HEAD_DIM, LT)),
            )
            state[lt, h, "zinvb"] = zinvb
        return go

    def piece_norm(lt, h):
        def go():
            ls = bass.ts(lt, LT)
            hd = bass.ts(h, HEAD_DIM)
            nc.vector.tensor_mul(
                out=attnTb[hd, ls],
                in0=attnTb[hd, ls],
                in1=state[lt, h, "zinvb"][hd, :],
            )
        return go

    def piece_recip_short(lt, h):
        # last-l-tile fast path: single-partition reciprocal, no DRAM bounce
        def go():
            zseg = state[lt, h, "zseg"]
            nc.vector.reciprocal(zseg[:], zseg[:])
            zsegb = zpool.tile([1, LT], BF16, tag="zsegb", name=f"zsegb_{lt}_{h}")
            nc.vector.tensor_copy(out=zsegb[:], in_=zseg[:])
            state[lt, h, "zsegb"] = zsegb
        return go

    def piece_bcast_pe(lt, h):
        # broadcast zinv across partitions with a ones-column matmul
        def go():
            zb = pso.tile([128, LT], F32, tag="pso", name=f"zb_{lt}_{h}")
            nc.tensor.matmul(
                zb[:], lhsT=onesb[:], rhs=state[lt, h, "zsegb"][:],
                start=True, stop=True,
            )
            state[lt, h, "zinvps"] = zb
        return go

    def piece_norm_short(lt, h):
        def go():
            ls = bass.ts(lt, LT)
            hd = bass.ts(h, HEAD_DIM)
            nc.vector.tensor_mul(
                out=attnTb[hd, ls],
                in0=attnTb[hd, ls],
                in1=state[lt, h, "zinvps"][hd, :],
            )
        return go

    def piece_outproj(lt, jt):
        def go():
            ls = bass.ts(lt, LT)
            ps = pso.tile([128, LT], F32, tag="pso", name="ps_out")
            nc.tensor.matmul(
                ps[:],
                lhsT=wos[:, bass.ts(jt, JT)],
                rhs=attnTb[:, ls],
                start=True,
                stop=True,
            )
            st = stage.tile([128, LT], F16, tag="st", name="st")
            if lt == NLT - 1 and jt % 2 == 0:
                # the exp stream is over by now: the idle scalar engine
                # takes half the tail's PSUM->f16 copies off the DVE
                nc.scalar.activation(
                    st[:], ps[:], mybir.ActivationFunctionType.Copy
                )
            else:
                nc.vector.tensor_copy(out=st[:], in_=ps[:])
            nc.gpsimd.dma_start(outT[bass.ts(jt, JT), ls], st[:])
        return go

    qk_proj(QTb, wqs, qTs, 0)

    # ---- attention blocks ----
    for b in range(NBLK):
        lt, h = divmod(b, HPC)
        ls = bass.ts(lt, LT)
        hd = bass.ts(h, HEAD_DIM)
        maskc = mask_tiles[b]
        pa = psa.tile([128, LT], F32, tag="psa", name=f"psa_{lt}_{h}")
        mt0 = 0
        prev_attn = None
        chunks_done = [0] if b == 0 else [NLT]
        for qi, qn in enumerate(QUADS):
            if b == 0:
                # interleave K/V projection chunks in consumption order
                need = min(NLT, (mt0 + qn + 3) // (LT // MT))
                while chunks_done[0] < need:
                    c = chunks_done[0]
                    qk_proj(KTb, wks, kTs, c)
                    vt_proj(c)
                    chunks_done[0] += 1
            if b == 1 and 1 <= qi <= 3:
                qk_proj(QTb, wqs, qTs, qi)  # PE filler + needed later
            if qi == 3 and b + 2 < NBLK:
                mask_fetch(b + 2, (NMT,))
            pop_pending()
            ss = psc.tile([128, QB, LT], F32, tag="psc", name="ss")
            for i in range(qn):
                nc.tensor.matmul(
                    ss[:, i, :],
                    lhsT=KTb[hd, bass.ts(mt0 + i, MT)],
                    rhs=QTb[hd, ls],
                    start=True,
                    stop=True,
                )
            # one-quad software pipeline on PE: the previous quad's attn
            # matmuls are emitted AFTER this quad's scores, so the in-order
            # PE queue never blocks scores behind exp->mask-mult
            if prev_attn is not None:
                prev_attn()
            pT = ppool.tile([128, QB, LT], BF16, tag="pT", name="pT")
            nc.scalar.activation(
                pT[:, :qn, :], ss[:, :qn, :], mybir.ActivationFunctionType.Exp
            )
            nc.vector.tensor_mul(
                out=pT[:, :qn, :],
                in0=pT[:, :qn, :],
                in1=maskc[:, mt0 : mt0 + qn, :],
            )

            def make_attn(mt0=mt0, qn=qn, pT=pT, pa=pa, h=h):
                def go():
                    for i in range(qn):
                        mt = mt0 + i
                        nc.tensor.matmul(
                            pa[:VROW, :],
                            lhsT=vaug[:, h, mt, :],
                            rhs=pT[:, i, :],
                            start=(mt == 0),
                            stop=(mt == NMT - 1),
                        )
                return go

            prev_attn = make_attn()
            mt0 += qn
            quad_no[0] += 1
        prev_attn()
        nc.vector.tensor_copy(out=attnTb[hd, ls], in_=pa[0:HEAD_DIM, :])
        zseg = zpool.tile([1, LT], F32, tag="zseg", name=f"zseg_{lt}_{h}")
        nc.vector.tensor_copy(
            out=zseg[:], in_=pa[HEAD_DIM : HEAD_DIM + 1, :]
        )
        state[lt, h, "zseg"] = zseg
        q0 = quad_no[0]
        if lt == NLT - 1:
            # short DMA-free chain so the final epilogue isn't
            # serialized behind 4 DMA-latency hops
            pending.append([q0 + 0, piece_recip_short(lt, h)])
            pending.append([q0 + 1, piece_bcast_pe(lt, h)])
            pending.append([q0 + 2, piece_norm_short(lt, h)])
        else:
            pending.append([q0 + 0, piece_zstore(lt, h)])
            pending.append([q0 + 1, piece_zload(lt, h)])
            pending.append([q0 + 2, piece_recip(lt, h)])
            pending.append([q0 + 3, piece_zbcast(lt, h)])
            pending.append([q0 + 4, piece_norm(lt, h)])
        if h == 1:
            for jt in range(NJT):
                pending.append([q0 + 4 + (jt + 1) // 2, piece_outproj(lt, jt)])

    pop_pending(budget=len(pending), flush=True)


_CACHE = {}


def _build():
    if "nc" in _CACHE:
        return _CACHE["nc"]
    nc = bacc.Bacc("TRN2", target_bir_lowering=False, debug=False,
                   num_devices=NCORES)
    qT = nc.dram_tensor("qT", [NLT, 128, NET, LT], BF16, kind="ExternalInput").ap()
    kT = nc.dram_tensor("kT", [NLT, 128, NET, LT], BF16, kind="ExternalInput").ap()
    vT = nc.dram_tensor("vT", [NLT, 128, NET, LT], BF16, kind="ExternalInput").ap()
    wqT = nc.dram_tensor("wqT", [128, NET, ROWS], BF16, kind="ExternalInput").ap()
    wkT = nc.dram_tensor("wkT", [128, NET, ROWS], BF16, kind="ExternalInput").ap()
    wvT = nc.dram_tensor("wvT", [128, NET, ROWS], BF16, kind="ExternalInput").ap()
    woT = nc.dram_tensor("woT", [ROWS, EMB], BF16, kind="ExternalInput").ap()
    maskP = nc.dram_tensor(
        "maskP", [NLT, HPC, 128, NMT, LT], FP8, kind="ExternalInput"
    ).ap()
    identT = nc.dram_tensor("identT", [128, 128], BF16, kind="ExternalInput").ap()
    outT = nc.dram_tensor("outT", [EMB, L], F16, kind="ExternalOutput").ap()

    with tile.TileContext(nc) as tc:
        _mha_kernel(tc, outT, qT, kT, vT, wqT, wkT, wvT, woT, maskP, identT)
    nc.compile()
    _CACHE["nc"] = nc
    return nc


def _pack_w(w):
    # [ROWS, EMB] -> w.T [EMB, ROWS] -> [128, NET, ROWS] with e = o*128+p
    return np.ascontiguousarray(
        w.T.reshape(NET, 128, ROWS).transpose(1, 0, 2)
    ).astype(NPBF16)


def _pack_mask(keep):
    # keep [HPC, l(query), m(key)] -> keepT [HPC, m, l]
    # -> [NLT, HPC, 128(p), NMT(mo), LT] chunk-contiguous, m = mo*128+p
    keepT = keep.swapaxes(1, 2)
    m5 = keepT.reshape(HPC, NMT, 128, NLT, LT).transpose(3, 0, 2, 1, 4)
    return np.ascontiguousarray(m5).astype(NPFP8)


def _pack_x(x):
    # [L, EMB] -> x.T [EMB, L] -> [NLT, 128(p), NET(o), LT] chunk-contiguous
    # (e = o*128+p) so every partition row of a chunk is one 8KB dense run
    return np.ascontiguousarray(
        x.T.reshape(NET, 128, NLT, LT).transpose(2, 1, 0, 3)
    ).astype(NPBF16)


def _prep_in_maps(q, k, v, mask, Wq, Wk, Wv, Wo):
    qT = _pack_x(q)
    kT = _pack_x(k)
    vT = _pack_x(v)
    in_maps = []
    for c in range(NCORES):
        rows = slice(c * ROWS, (c + 1) * ROWS)
        in_maps.append({
            "qT": qT,
            "kT": kT,
            "vT": vT,
            "wqT": _pack_w(Wq[rows] * SCALE),
            "wkT": _pack_w(Wk[rows]),
            "wvT": _pack_w(Wv[rows]),
            "woT": np.ascontiguousarray(Wo[:, rows].T).astype(NPBF16),
            "maskP": _pack_mask(~mask[c * HPC : (c + 1) * HPC]),
            "identT": np.eye(128, dtype=NPBF16),
        })
    return in_maps


def run(q, k, v, mask, Wq, Wk, Wv, Wo, **spmd_kwargs):
    nc = _build()
    in_maps = _prep_in_maps(q, k, v, mask, Wq, Wk, Wv, Wo)
    res = run_bass_kernel_spmd(nc, in_maps, list(range(NCORES)), **spmd_kwargs)
    outT = np.zeros((EMB, L), np.float64)
    for r in res.results:
        outT += r["outT"].astype(np.float64)
    out = np.ascontiguousarray(outT.T).astype(np.float32)
    return out, res


def kernel(q, k, v, mask, Wq, Wk, Wv, Wo):
    q, k, v = (np.asarray(x, np.float32) for x in (q, k, v))
    Wq, Wk, Wv, Wo = (np.asarray(x, np.float32) for x in (Wq, Wk, Wv, Wo))
    mask = np.asarray(mask, bool)
    out, _ = run(q, k, v, mask, Wq, Wk, Wv, Wo)
    return out


# revision 25
# speedup vs baseline: 1.2829x; 1.2246x over previous
"""Multi-head attention (L=2048, EMB=1024, H=16, D=64) on 8 TRN2 NeuronCores.

Tensor-parallel over heads: core i owns heads {2i, 2i+1} (a 128-row block of
Wq/Wk/Wv and a 128-column block of Wo). Each core computes its two heads'
attention plus its partial output projection; the host sums the 8 partials.

Device-side layout is fully transposed (scores^T = [m, l]) so no on-device
transposes of the score matrix are needed:
  QT[d, l] = (Wq_shard @ q^T)        lhsT = (Wq_shard/8)^T, rhs = q^T
  KT[d, l] = (Wk_shard @ k^T)
  VT[d, m] = (Wv_shard @ v^T)        then XBAR DMA-transpose -> vaug[m, d]
  sT[m, l] = KT_h^T @ QT_h           (per head, contraction d=64)
  pT       = exp(sT) * keepT         (no max-subtraction: |s| <~ 9)
  attnT/Z  = [V_h | 1]^T @ pT        (ones column gives softmax denominator)
  outT     = Wo_shard^T-block @ (attnT / Z)   partial, summed on host

Key optimizations over the straightforward version (from NTFF traces):
- mask ships as fp8 (0.0/1.0) in a chunk-contiguous DRAM layout and is
  cast to bf16 in-flight by the gpsimd SWDGE ring: halves mask HBM reads
  and makes every DMA packet dense.
- output partials ship as f16 (halves output traffic; ~0.05% noise).
- V projection computes VT with 32 big matmuls + 16 XBAR DMA transposes
  instead of 128 tiny matmuls (PE time for V drops ~4x).
- input DMAs are issued in exact consumption order across the three
  rings (scalar/sync HWDGE + gpsimd SWDGE) so the first score matmul
  isn't stuck behind bulk traffic.
- per-(lt,h) softmax-denominator chains (DRAM bounce to spread the
  reciprocal over 8 partitions) and the output projection are deferred
  and drip-fed into the following block's quad stream; the final
  block's epilogue is flushed ASAP to keep the tail short.
- PSUM: 6 banks double-buffered scores, 1 bank attention accumulator,
  1 bank shared by projections + output projection.
"""

import sys

for _p in ("/opt/trn_rl_repo",):
    if _p not in sys.path:
        sys.path.insert(0, _p)

from contextlib import ExitStack

import ml_dtypes
import numpy as np

import concourse.bass as bass
import concourse.tile as tile
from concourse import bacc, mybir
from concourse._compat import with_exitstack
from concourse.bass_utils import run_bass_kernel_spmd

BF16 = mybir.dt.bfloat16
FP8 = mybir.dt.float8e4
F16 = mybir.dt.float16
F32 = mybir.dt.float32
NPBF16 = ml_dtypes.bfloat16
NPFP8 = ml_dtypes.float8_e4m3
NPF16 = np.float16

L = 2048
EMB = 1024
NHEAD = 16
HEAD_DIM = 64
NCORES = 8
HPC = NHEAD // NCORES  # heads per core = 2
ROWS = HPC * HEAD_DIM  # weight rows per core = 128
SCALE = HEAD_DIM ** -0.5

LT = 512               # l-tile (matmul free dim / PSUM bank)
NLT = L // LT          # 4
MT = 128               # m-tile (key-block on partitions)
NMT = L // MT          # 16
ET = 128               # contraction tile over EMB
NET = EMB // ET        # 8
JT = 128               # output-row tile
NJT = EMB // JT        # 8
NBLK = NLT * HPC       # 8 (lt, h) blocks

QUADS = (3, 3, 3, 3, 2, 2)   # m-tiles per exp/mask-mult instruction
QB = 3                        # psc tile m-capacity (PSUM banks per slot)
VROW = 66                     # 64 V cols + ones + pad


@with_exitstack
def _mha_kernel(ctx, tc, outT, qT, kT, vT, wqT, wkT, wvT, woT, maskP, identT):
    nc = tc.nc

    const = ctx.enter_context(tc.tile_pool(name="const", bufs=1))
    ppool = ctx.enter_context(tc.tile_pool(name="ptiles", bufs=4))
    maskp = ctx.enter_context(tc.tile_pool(name="maskp", bufs=3))
    stage = ctx.enter_context(tc.tile_pool(name="stage", bufs=4))
    zpool = ctx.enter_context(tc.tile_pool(name="zpool", bufs=2))
    psc = ctx.enter_context(tc.tile_pool(name="psc", bufs=2, space="PSUM"))
    psa = ctx.enter_context(tc.tile_pool(name="psa", bufs=1, space="PSUM"))
    pso = ctx.enter_context(tc.tile_pool(name="pso", bufs=1, space="PSUM"))

    # ---- resident input tiles ----
    qTs = const.tile([128, NET, L], BF16, tag="qTs")
    kTs = const.tile([128, NET, L], BF16, tag="kTs")
    vTs = const.tile([128, NET, L], BF16, tag="vTs")
    wqs = const.tile([128, NET, ROWS], BF16, tag="wqs")
    wks = const.tile([128, NET, ROWS], BF16, tag="wks")
    wvs = const.tile([128, NET, ROWS], BF16, tag="wvs")
    wos = const.tile([128, EMB], BF16, tag="wos")  # [hd, j]
    # inputs arrive chunk-contiguous ([NLT, 128, NET, LT] in DRAM) so every
    # DMA packet is a dense 2KB run
    def chunk(eng, dst, src4, lc):
        eng.dma_start(dst[:, :, bass.ts(lc, LT)], src4[lc])

    # ---- input DMA issue, consumption order ----
    # Only dependency-free DRAM->SBUF loads go on the scalar/sync HWDGE
    # rings: a data-dependent dma_start would block the issuing ENGINE
    # (scalar runs exp; sync runs the z-chains) at its semaphore wait.
    # The issuing ENGINE is occupied for the whole transfer of each DMA it
    # starts, and concurrent rings do NOT share HBM fairly (the gpsimd ring
    # dominates). So ALL order-sensitive input loads ride the single gpsimd
    # ring in exact consumption order; scalar (exp) issues nothing; sync
    # (z-chains from ~45us) issues nothing at startup.
    identb = const.tile([128, 128], BF16, tag="identb")
    nc.gpsimd.dma_start(wqs[:], wqT[:])
    chunk(nc.gpsimd, qTs, qT, 0)
    nc.gpsimd.dma_start(wks[:], wkT[:])
    chunk(nc.gpsimd, kTs, kT, 0)
    nc.gpsimd.dma_start(wvs[:], wvT[:])
    nc.gpsimd.dma_start(identb[:], identT[:])
    chunk(nc.gpsimd, vTs, vT, 0)

    # gpsimd SWDGE: mask chunks (fp8 DRAM -> bf16 SBUF casting DMAs)
    mask_tiles = {}

    def mask_fetch(b, splits):
        lt, h = divmod(b, HPC)
        mc = maskp.tile([128, NMT, LT], BF16, tag="maskc", name=f"maskc_{lt}_{h}")
        mask_tiles[b] = mc
        a = 0
        for n in splits:
            nc.gpsimd.dma_start(
                mc[:, a : a + n, :], maskP[lt, h, :, a : a + n, :]
            )
            a += n

    # first-block mask sub-chunks interleaved with the k/v chunk stream,
    # all in the order block 0's quads consume them
    mc0 = maskp.tile([128, NMT, LT], BF16, tag="maskc", name="maskc_0_0")
    mask_tiles[0] = mc0

    def m0_sub(a, b):
        nc.gpsimd.dma_start(mc0[:, a:b, :], maskP[0, 0, :, a:b, :])

    m0_sub(0, 3)
    chunk(nc.gpsimd, kTs, kT, 1)
    chunk(nc.gpsimd, vTs, vT, 1)
    m0_sub(3, 6)
    chunk(nc.gpsimd, kTs, kT, 2)
    chunk(nc.gpsimd, vTs, vT, 2)
    m0_sub(6, 9)
    chunk(nc.gpsimd, kTs, kT, 3)
    chunk(nc.gpsimd, vTs, vT, 3)
    m0_sub(9, 12)
    m0_sub(12, 16)
    mask_fetch(1, (8, 8))
    chunk(nc.gpsimd, qTs, qT, 1)
    chunk(nc.gpsimd, qTs, qT, 2)
    chunk(nc.gpsimd, qTs, qT, 3)
    nc.gpsimd.dma_start(wos[:], woT[:])

    # ---- working tiles ----
    QTb = const.tile([128, L], BF16, tag="QTb")
    KTb = const.tile([128, L], BF16, tag="KTb")
    VTb = const.tile([128, L], BF16, tag="VTb")
    vaug = const.tile([128, HPC, NMT, VROW], BF16, tag="vaug")
    nc.vector.memset(vaug[:, :, :, HEAD_DIM : HEAD_DIM + 1], 1.0)
    nc.vector.memset(vaug[:, :, :, HEAD_DIM + 1 : VROW], 0.0)
    attnTb = const.tile([128, L], BF16, tag="attnTb")
    onesb = const.tile([1, 128], BF16, tag="onesb")
    nc.vector.memset(onesb[:], 1.0)

    def qk_proj(dst, w, x, lc):
        ps = pso.tile([128, LT], F32, tag="pso", name="ps_proj")
        for et in range(NET):
            nc.tensor.matmul(
                ps[:],
                lhsT=w[:, et, :],
                rhs=x[:, et, bass.ts(lc, LT)],
                start=(et == 0),
                stop=(et == NET - 1),
            )
        nc.vector.tensor_copy(out=dst[:, bass.ts(lc, LT)], in_=ps[:])

    def vt_proj(lc):
        qk_proj(VTb, wvs, vTs, lc)
        for mi in range(LT // MT):
            mt = lc * (LT // MT) + mi
            # PE transpose VT[d2h, m-tile] -> [m, d2h]; reuses the idle
            # outproj PSUM bank (outproj only starts at block 2)
            trp = pso.tile([128, MT], BF16, tag="pso", name=f"tr_{mt}")
            nc.tensor.transpose(trp[:], VTb[:, bass.ts(mt, MT)], identb[:])
            for h in range(HPC):
                nc.vector.tensor_copy(
                    out=vaug[:, h, mt, 0:HEAD_DIM],
                    in_=trp[:, bass.ts(h, HEAD_DIM)],
                )

    # ---- deferred epilogue pieces, drip-fed into later quads ----
    zdram = nc.dram_tensor("zdram", [NLT, HPC, LT], F32).ap()
    zidram = nc.dram_tensor("zidram", [NLT, HPC, LT], BF16).ap()
    state = {}
    pending = []  # [ready_quad, fn]
    quad_no = [0]

    def pop_pending(budget=3, flush=False):
        while pending and budget > 0:
            if not flush and pending[0][0] > quad_no[0]:
                break
            pending.pop(0)[1]()
            budget -= 1

    def piece_zstore(lt, h):
        def go():
            nc.sync.dma_start(
                zdram[lt, h][None, :], state[lt, h, "zseg"][:]
            )
        return go

    def piece_zload(lt, h):
        def go():
            zsp = zpool.tile([8, LT // 8], F32, tag="zsp", name=f"zsp_{lt}_{h}")
            nc.sync.dma_start(zsp[:], zdram[lt, h].rearrange("(o p) -> o p", o=8))
            state[lt, h, "zsp"] = zsp
        return go

    def piece_recip(lt, h):
        def go():
            zsp = state[lt, h, "zsp"]
            nc.vector.reciprocal(zsp[:], zsp[:])
            zspb = zpool.tile([8, LT // 8], BF16, tag="zspb", name=f"zspb_{lt}_{h}")
            nc.vector.tensor_copy(out=zspb[:], in_=zsp[:])
            nc.sync.dma_start(
                zidram[lt, h].rearrange("(o p) -> o p", o=8), zspb[:]
            )
        return go

    def piece_zbcast(lt, h):
        def go():
            # full-height tile so the norm's operands share a base partition
            zinvb = zpool.tile(
                [128, LT], BF16, tag="zinvb", name=f"zinvb_{lt}_{h}"
            )
            nc.sync.dma_start(
                zinvb[bass.ts(h, HEAD_DIM), :],
                zidram[lt, h][None, :].to_broadcast((HEAD_DIM, LT)),
            )
            state[lt, h, "zinvb"] = zinvb
        return go

    def piece_norm(lt, h):
        def go():
            ls = bass.ts(lt, LT)
            hd = bass.ts(h, HEAD_DIM)
            nc.vector.tensor_mul(
                out=attnTb[hd, ls],
                in0=attnTb[hd, ls],
                in1=state[lt, h, "zinvb"][hd, :],
            )
        return go

    def piece_recip_short(lt, h):
        # last-l-tile fast path: single-partition reciprocal, no DRAM bounce
        def go():
            zseg = state[lt, h, "zseg"]
            nc.vector.reciprocal(zseg[:], zseg[:])
            zsegb = zpool.tile([1, LT], BF16, tag="zsegb", name=f"zsegb_{lt}_{h}")
            nc.vector.tensor_copy(out=zsegb[:], in_=zseg[:])
            state[lt, h, "zsegb"] = zsegb
        return go

    def piece_bcast_pe(lt, h):
        # broadcast zinv across partitions with a ones-column matmul
        def go():
            zb = pso.tile([128, LT], F32, tag="pso", name=f"zb_{lt}_{h}")
            nc.tensor.matmul(
                zb[:], lhsT=onesb[:], rhs=state[lt, h, "zsegb"][:],
                start=True, stop=True,
            )
            state[lt, h, "zinvps"] = zb
        return go

    def piece_norm_short(lt, h):
        def go():
            ls = bass.ts(lt, LT)
            hd = bass.ts(h, HEAD_DIM)
            nc.vector.tensor_mul(
                out=attnTb[hd, ls],
                in0=attnTb[hd, ls],
                in1=state[lt, h, "zinvps"][hd, :],
            )
        return go

    def piece_outproj(lt, jt):
        def go():
            ls = bass.ts(lt, LT)
            ps = pso.tile([128, LT], F32, tag="pso", name="ps_out")
            nc.tensor.matmul(
                ps[:],
                lhsT=wos[:, bass.ts(jt, JT)],
                rhs=attnTb[:, ls],
                start=True,
                stop=True,
            )
            st = stage.tile([128, LT], F16, tag="st", name="st")
            if lt == NLT - 1 and jt % 2 == 0:
                # the exp stream is over by now: the idle scalar engine
                # takes half the tail's PSUM->f16 copies off the DVE
                nc.scalar.activation(
                    st[:], ps[:], mybir.ActivationFunctionType.Copy
                )
            else:
                nc.vector.tensor_copy(out=st[:], in_=ps[:])
            nc.gpsimd.dma_start(outT[bass.ts(jt, JT), ls], st[:])
        return go

    qk_proj(QTb, wqs, qTs, 0)

    # ---- attention blocks ----
    for b in range(NBLK):
        lt, h = divmod(b, HPC)
        ls = bass.ts(lt, LT)
        hd = bass.ts(h, HEAD_DIM)
        maskc = mask_tiles[b]
        pa = psa.tile([128, LT], F32, tag="psa", name=f"psa_{lt}_{h}")
        mt0 = 0
        prev_attn = None
        chunks_done = [0] if b == 0 else [NLT]
        for qi, qn in enumerate(QUADS):
            if b == 0:
                # interleave K/V projection chunks in consumption order
                need = min(NLT, (mt0 + qn + 3) // (LT // MT))
                while chunks_done[0] < need:
                    c = chunks_done[0]
                    qk_proj(KTb, wks, kTs, c)
                    vt_proj(c)
                    chunks_done[0] += 1
            if b == 1 and 1 <= qi <= 3:
                qk_proj(QTb, wqs, qTs, qi)  # PE filler + needed later
            if qi == 3 and b + 2 < NBLK:
                mask_fetch(b + 2, (NMT,))
            pop_pending()
            ss = psc.tile([128, QB, LT], F32, tag="psc", name="ss")
            for i in range(qn):
                nc.tensor.matmul(
                    ss[:, i, :],
                    lhsT=KTb[hd, bass.ts(mt0 + i, MT)],
                    rhs=QTb[hd, ls],
                    start=True,
                    stop=True,
                )
            # one-quad software pipeline on PE: the previous quad's attn
            # matmuls are emitted AFTER this quad's scores, so the in-order
            # PE queue never blocks scores behind exp->mask-mult
            if prev_attn is not None:
                prev_attn()
            pT = ppool.tile([128, QB, LT], BF16, tag="pT", name="pT")
            nc.scalar.activation(
                pT[:, :qn, :], ss[:, :qn, :], mybir.ActivationFunctionType.Exp
            )
            nc.vector.tensor_mul(
                out=pT[:, :qn, :],
                in0=pT[:, :qn, :],
                in1=maskc[:, mt0 : mt0 + qn, :],
            )

            def make_attn(mt0=mt0, qn=qn, pT=pT, pa=pa, h=h):
                def go():
                    for i in range(qn):
                        mt = mt0 + i
                        nc.tensor.matmul(
                            pa[:VROW, :],
                            lhsT=vaug[:, h, mt, :],
                            rhs=pT[:, i, :],
                            start=(mt == 0),
                            stop=(mt == NMT - 1),
                        )
                return go

            prev_attn = make_attn()
            mt0 += qn
            quad_no[0] += 1
        prev_attn()
        nc.vector.tensor_copy(out=attnTb[hd, ls], in_=pa[0:HEAD_DIM, :])
        zseg = zpool.tile([1, LT], F32, tag="zseg", name=f"zseg_{lt}_{h}")
        nc.vector.tensor_copy(
            out=zseg[:], in_=pa[HEAD_DIM : HEAD_DIM + 1, :]
        )
        state[lt, h, "zseg"] = zseg
        q0 = quad_no[0]
        if lt == NLT - 1:
            # short DMA-free chain so the final epilogue isn't
            # serialized behind 4 DMA-latency hops
            pending.append([q0 + 0, piece_recip_short(lt, h)])
            pending.append([q0 + 1, piece_bcast_pe(lt, h)])
            pending.append([q0 + 2, piece_norm_short(lt, h)])
        else:
            pending.append([q0 + 0, piece_zstore(lt, h)])
            pending.append([q0 + 1, piece_zload(lt, h)])
            pending.append([q0 + 2, piece_recip(lt, h)])
            pending.append([q0 + 3, piece_zbcast(lt, h)])
            pending.append([q0 + 4, piece_norm(lt, h)])
        if h == 1:
            for jt in range(NJT):
                pending.append([q0 + 4 + (jt + 1) // 2, piece_outproj(lt, jt)])

    pop_pending(budget=len(pending), flush=True)


_CACHE = {}


def _build():
    if "nc" in _CACHE:
        return _CACHE["nc"]
    nc = bacc.Bacc("TRN2", target_bir_lowering=False, debug=False,
                   num_devices=NCORES)
    qT = nc.dram_tensor("qT", [NLT, 128, NET, LT], BF16, kind="ExternalInput").ap()
    kT = nc.dram_tensor("kT", [NLT, 128, NET, LT], BF16, kind="ExternalInput").ap()
    vT = nc.dram_tensor("vT", [NLT, 128, NET, LT], BF16, kind="ExternalInput").ap()
    wqT = nc.dram_tensor("wqT", [128, NET, ROWS], BF16, kind="ExternalInput").ap()
    wkT = nc.dram_tensor("wkT", [128, NET, ROWS], BF16, kind="ExternalInput").ap()
    wvT = nc.dram_tensor("wvT", [128, NET, ROWS], BF16, kind="ExternalInput").ap()
    woT = nc.dram_tensor("woT", [ROWS, EMB], BF16, kind="ExternalInput").ap()
    maskP = nc.dram_tensor(
        "maskP", [NLT, HPC, 128, NMT, LT], FP8, kind="ExternalInput"
    ).ap()
    identT = nc.dram_tensor("identT", [128, 128], BF16, kind="ExternalInput").ap()
    outT = nc.dram_tensor("outT", [EMB, L], F16, kind="ExternalOutput").ap()

    with tile.TileContext(nc) as tc:
        _mha_kernel(tc, outT, qT, kT, vT, wqT, wkT, wvT, woT, maskP, identT)
    nc.compile()
    _CACHE["nc"] = nc
    return nc


def _pack_w(w):
    # [ROWS, EMB] -> w.T [EMB, ROWS] -> [128, NET, ROWS] with e = o*128+p
    return np.ascontiguousarray(
        w.T.reshape(NET, 128, ROWS).transpose(1, 0, 2)
    ).astype(NPBF16)


def _pack_mask(keep):
    # keep [HPC, l(query), m(key)] -> keepT [HPC, m, l]
    # -> [NLT, HPC, 128(p), NMT(mo), LT] chunk-contiguous, m = mo*128+p
    keepT = keep.swapaxes(1, 2)
    m5 = keepT.reshape(HPC, NMT, 128, NLT, LT).transpose(3, 0, 2, 1, 4)
    return np.ascontiguousarray(m5).astype(NPFP8)


def _pack_x(x):
    # [L, EMB] -> x.T [EMB, L] -> [NLT, 128(p), NET(o), LT] chunk-contiguous
    # (e = o*128+p) so every partition row of a chunk is one 8KB dense run
    return np.ascontiguousarray(
        x.T.reshape(NET, 128, NLT, LT).transpose(2, 1, 0, 3)
    ).astype(NPBF16)


def _prep_in_maps(q, k, v, mask, Wq, Wk, Wv, Wo):
    qT = _pack_x(q)
    kT = _pack_x(k)
    vT = _pack_x(v)
    in_maps = []
    for c in range(NCORES):
        rows = slice(c * ROWS, (c + 1) * ROWS)
        in_maps.append({
            "qT": qT,
            "kT": kT,
            "vT": vT,
            "wqT": _pack_w(Wq[rows] * SCALE),
            "wkT": _pack_w(Wk[rows]),
            "wvT": _pack_w(Wv[rows]),
            "woT": np.ascontiguousarray(Wo[:, rows].T).astype(NPBF16),
            "maskP": _pack_mask(~mask[c * HPC : (c + 1) * HPC]),
            "identT": np.eye(128, dtype=NPBF16),
        })
    return in_maps


def run(q, k, v, mask, Wq, Wk, Wv, Wo, **spmd_kwargs):
    nc = _build()
    in_maps = _prep_in_maps(q, k, v, mask, Wq, Wk, Wv, Wo)
    res = run_bass_kernel_spmd(nc, in_maps, list(range(NCORES)), **spmd_kwargs)
    outT = np.zeros((EMB, L), np.float64)
    for r in res.results:
        outT += r["outT"].astype(np.float64)
    out = np.ascontiguousarray(outT.T).astype(np.float32)
    return out, res


def kernel(q, k, v, mask, Wq, Wk, Wv, Wo):
    q, k, v = (np.asarray(x, np.float32) for x in (q, k, v))
    Wq, Wk, Wv, Wo = (np.asarray(x, np.float32) for x in (Wq, Wk, Wv, Wo))
    mask = np.asarray(mask, bool)
    out, _ = run(q, k, v, mask, Wq, Wk, Wv, Wo)
    return out
